# revision 7
# baseline (speedup 1.0000x reference)
"""TRN2 Bass kernel for the BEATs-style VQ tokenizer (vq_codebook problem).

Data-parallel over batch B=8 across 8 NeuronCores. Each core processes its
1024-token slice end to end:

  phase A: gate (token layout, bf16x3 matmuls + DVE polynomial exp softmax),
           three expert matmuls in transposed [d, token] layout (bf16x3),
           gated combine via PE broadcast-transposes + DVE, vehicle-mean add,
           then split enhancedT into bf16 hi/lo for the logits matmuls.
  phase B: logits = enhancedT.T @ W_logits as bf16x3 (hi*hi + hi*lo + lo*hi),
           tiled [128 tok, 512 cb] psum tiles; per tile: ScalarE copy to SBUF
           (-> HBM logits out), ScalarE exp (fp16, accum row-sums), VectorE
           block-max. Two sweeps of W_logits (4 token blocks each) keep the
           fp16 exp buffer within SBUF.
  finalize per token block: softmax scale (Newton-refined reciprocal),
           exact argmax (block max -> winning block gather from HBM ->
           position via iota/is_equal), codebook gather, vq-loss partials.

Precision: all matmuls are bf16 hi/lo split x3 (exact products, f32 psum
accumulation) giving ~4e-5 absmax logit error vs the f32 reference, far under
the 1.8e-5..~0.1 top-2 logit gaps -> argmax matches the reference exactly.
quantized == codebook[idx] holds bitwise in the reference (verified).
"""

import numpy as np
import ml_dtypes

import concourse.bass as bass
import concourse.bacc as bacc
import concourse.mybir as mybir
import concourse.tile as tile
from concourse.tile_rust import add_dep_helper

B, T, D, KCB = 8, 1024, 768, 8192
NTB = 8          # token blocks of 128 per core
NNB = 16         # codebook-dim blocks of 512
DC = 6           # d chunks of 128
NSW = 2          # W_logits sweeps
TB_PER_SW = NTB // NSW

f32 = mybir.dt.float32
f16 = mybir.dt.float16
bf16 = mybir.dt.bfloat16
i32 = mybir.dt.int32
AX = mybir.AxisListType.X
OP = mybir.AluOpType
AF = mybir.ActivationFunctionType

_BF = ml_dtypes.bfloat16


def _exp_poly_coeffs():
    """Power-basis coeffs (in u = x/4 + 2, u in [-2,2]) approximating
    e^(u-2); gate exp is then poly(u)^4 = e^x for x in [-16, 0]."""
    u = np.linspace(-2, 2, 20001)
    cheb = np.polynomial.chebyshev.Chebyshev.fit(u, np.exp(u - 2.0), deg=15)
    c = cheb.convert(kind=np.polynomial.Polynomial).coef
    rel = np.abs(np.polyval(c[::-1], u) / np.exp(u - 2.0) - 1.0).max()
    assert rel < 1e-7, rel
    return [float(v) for v in c]  # c[k] multiplies u^k

_EXP_C = _exp_poly_coeffs()


def build_nc(with_bias: bool):
    nc = bacc.Bacc(None)

    # ---- inputs (per core) ----
    xTh = nc.dram_tensor("xTh", [D, T], bf16, kind="ExternalInput")
    xTl = nc.dram_tensor("xTl", [D, T], bf16, kind="ExternalInput")
    Wh = [nc.dram_tensor(f"W{e}h", [D, D], bf16, kind="ExternalInput") for e in range(3)]
    Wl = [nc.dram_tensor(f"W{e}l", [D, D], bf16, kind="ExternalInput") for e in range(3)]
    Wgh = nc.dram_tensor("Wgh", [D, 3], bf16, kind="ExternalInput")
    Wgl = nc.dram_tensor("Wgl", [D, 3], bf16, kind="ExternalInput")
    WLh = nc.dram_tensor("WLh", [D, KCB], bf16, kind="ExternalInput")
    WLl = nc.dram_tensor("WLl", [D, KCB], bf16, kind="ExternalInput")
    vbar = nc.dram_tensor("vbar", [128, DC], f32, kind="ExternalInput")
    cb = nc.dram_tensor("cb", [KCB, D], f32, kind="ExternalInput")
    if with_bias:
        bexph = nc.dram_tensor("bexph", [3, D], bf16, kind="ExternalInput")
        bexpl = nc.dram_tensor("bexpl", [3, D], bf16, kind="ExternalInput")
        bgh = nc.dram_tensor("bgh", [1, 3], bf16, kind="ExternalInput")
        bgl = nc.dram_tensor("bgl", [1, 3], bf16, kind="ExternalInput")
        blh = nc.dram_tensor("blh", [1, KCB], bf16, kind="ExternalInput")
        bll = nc.dram_tensor("bll", [1, KCB], bf16, kind="ExternalInput")

    # ---- outputs (per core) ----
    logits_o = nc.dram_tensor("logits_o", [T, KCB], f32, kind="ExternalOutput")
    soft_o = nc.dram_tensor("soft_o", [T, KCB], f32, kind="ExternalOutput")
    quant_o = nc.dram_tensor("quant_o", [T, D], f32, kind="ExternalOutput")
    idx_o = nc.dram_tensor("idx_o", [NTB, 128], f32, kind="ExternalOutput")
    loss_o = nc.dram_tensor("loss_o", [128, 1], f32, kind="ExternalOutput")

    logits_rows = logits_o[:, :].rearrange("t (b c) -> (t b) c", c=512)

    with tile.TileContext(nc) as tc:
        with tc.tile_pool(name="pers", bufs=1) as pers:
            # persistent tiles
            EhT = pers.tile([128, DC, T], bf16)
            ElT = pers.tile([128, DC, T], bf16)
            spart = pers.tile([128, NTB, NNB], f32)
            mpart = pers.tile([128, NTB, NNB], f32)
            lossp = pers.tile([128, NTB * DC], f32)
            iota512 = pers.tile([128, 512], f32)
            iota16 = pers.tile([128, NNB], f32)
            ident = pers.tile([128, 128], f32)
            vbar_t = pers.tile([128, DC], f32)
            tokio = pers.tile([128, NTB], f32)
            ones_h = pers.tile([1, T], bf16)

            # constants
            it512 = pers.tile([128, 512], i32)
            nc.gpsimd.iota(it512, pattern=[[-1, 512]], base=511, channel_multiplier=0)
            nc.vector.tensor_copy(out=iota512, in_=it512)
            it16 = pers.tile([128, NNB], i32)
            nc.gpsimd.iota(it16, pattern=[[-1, NNB]], base=NNB - 1, channel_multiplier=0)
            nc.vector.tensor_copy(out=iota16, in_=it16)
            itok = pers.tile([128, NTB], i32)
            nc.gpsimd.iota(itok, pattern=[[128, NTB]], base=0, channel_multiplier=1)
            nc.vector.tensor_copy(out=tokio, in_=itok)
            from concourse.masks import make_identity
            make_identity(nc, ident)
            nc.sync.dma_start(out=vbar_t, in_=vbar[:, :])
            nc.vector.memset(ones_h, 1.0)

            # =========================== phase A ===========================
            with tc.tile_pool(name="pA", bufs=1) as pA:
                xh_t = pA.tile([128, DC, T], bf16)
                nc.sync.dma_start(out=xh_t, in_=xTh[:, :].rearrange("(c p) t -> p c t", p=128))
                xl_t = pA.tile([128, DC, T], bf16)
                nc.sync.dma_start(out=xl_t, in_=xTl[:, :].rearrange("(c p) t -> p c t", p=128))
                wgh_t = pA.tile([128, DC, 3], bf16)
                nc.sync.dma_start(out=wgh_t, in_=Wgh[:, :].rearrange("(c p) n -> p c n", p=128))
                wgl_t = pA.tile([128, DC, 3], bf16)
                nc.sync.dma_start(out=wgl_t, in_=Wgl[:, :].rearrange("(c p) n -> p c n", p=128))
                if with_bias:
                    bgh_t = pA.tile([1, 3], bf16)
                    nc.sync.dma_start(out=bgh_t, in_=bgh[:, :])
                    bgl_t = pA.tile([1, 3], bf16)
                    nc.sync.dma_start(out=bgl_t, in_=bgl[:, :])
                    xon = pA.tile([1, T], bf16)
                    nc.vector.memset(xon, 1.0)
                    bexph_t = pA.tile([3, D], bf16)
                    nc.sync.dma_start(out=bexph_t, in_=bexph[:, :])
                    bexpl_t = pA.tile([3, D], bf16)
                    nc.sync.dma_start(out=bexpl_t, in_=bexpl[:, :])

                # ---- gate logits, token layout [128 tok, 3] per tb ----
                glog = pA.tile([128, NTB, 3], f32)
                with tc.tile_pool(name="gps", bufs=2, space="PSUM") as gps:
                    for tb in range(NTB):
                        g_ps = gps.tile([128, 3], f32, tag="g")
                        first = True
                        for si, (xs, ws) in enumerate(((xh_t, wgh_t), (xh_t, wgl_t), (xl_t, wgh_t))):
                            for c in range(DC):
                                last = (not with_bias) and si == 2 and c == DC - 1
                                nc.tensor.matmul(
                                    out=g_ps[:, :],
                                    lhsT=xs[:, c, tb * 128:(tb + 1) * 128],
                                    rhs=ws[:, c, :],
                                    start=first, stop=last)
                                first = False
                        if with_bias:
                            nc.tensor.matmul(out=g_ps[:, :], lhsT=xon[0:1, tb * 128:(tb + 1) * 128],
                                             rhs=bgh_t[0:1, :], start=False, stop=False)
                            nc.tensor.matmul(out=g_ps[:, :], lhsT=xon[0:1, tb * 128:(tb + 1) * 128],
                                             rhs=bgl_t[0:1, :], start=False, stop=True)
                        nc.scalar.copy(out=glog[:, tb, :], in_=g_ps[:, :])

                # ---- gate softmax: shift by max, DVE polynomial exp ----
                cent = pA.tile([128, NTB, 3], f32)
                for tb in range(NTB):
                    nm = pA.tile([128, 1], f32, tag="gnm")
                    nc.vector.tensor_reduce(out=nm, in_=glog[:, tb, :], axis=AX,
                                            op=OP.max, negate=True)
                    nc.vector.tensor_scalar(out=cent[:, tb, :], in0=glog[:, tb, :],
                                            scalar1=nm, scalar2=None, op0=OP.add)
                flat = cent[:, :, :].rearrange("p a b -> p (a b)")
                u = pA.tile([128, NTB * 3], f32)
                # u = max(x, -16) * 0.25 + 2
                nc.vector.tensor_scalar(out=u, in0=flat, scalar1=-16.0, scalar2=0.25,
                                        op0=OP.max, op1=OP.mult)
                nc.vector.tensor_scalar(out=u, in0=u, scalar1=2.0, scalar2=None, op0=OP.add)
                pv = pA.tile([128, NTB * 3], f32)
                nc.vector.memset(pv, 0.0)
                for k in range(15, 0, -1):
                    nc.vector.scalar_tensor_tensor(out=pv, in0=pv, scalar=float(_EXP_C[k]),
                                                   in1=u, op0=OP.add, op1=OP.mult)
                nc.vector.tensor_scalar(out=pv, in0=pv, scalar1=float(_EXP_C[0]),
                                        scalar2=None, op0=OP.add)
                nc.vector.tensor_tensor(out=pv, in0=pv, in1=pv, op=OP.mult)
                nc.vector.tensor_tensor(out=pv, in0=pv, in1=pv, op=OP.mult)
                gexp = pv.rearrange("p (a b) -> p a b", b=3)

                # row sums + newton reciprocal -> rs [128, NTB]
                gs = pA.tile([128, NTB], f32)
                nc.vector.tensor_reduce(out=gs, in_=gexp, axis=AX, op=OP.add)
                rs0 = pA.tile([128, NTB], f32)
                nc.vector.reciprocal(out=rs0, in_=gs)
                tnw = pA.tile([128, NTB], f32)
                nc.vector.tensor_tensor(out=tnw, in0=gs, in1=rs0, op=OP.mult)
                nc.vector.tensor_scalar(out=tnw, in0=tnw, scalar1=2.0, scalar2=-1.0,
                                        op0=OP.subtract, op1=OP.mult)
                rs = pA.tile([128, NTB], f32)
                nc.vector.tensor_tensor(out=rs, in0=rs0, in1=tnw, op=OP.mult)

                # ---- broadcast g0/g1/g2 and 1/s across partitions ----
                gb_sb = [pA.tile([128, T], f32, tag=f"gb{e}", name=f"gb{e}") for e in range(3)]
                rb_sb = pA.tile([128, T], f32, tag="rb")
                with tc.tile_pool(name="bps", bufs=3, space="PSUM") as bps:
                    for tb in range(NTB):
                        for e in range(3):
                            t_ps = bps.tile([128, 128], f32, tag="bc")
                            nc.tensor.transpose(
                                out=t_ps[:, :],
                                in_=gexp[:, tb, e:e + 1].to_broadcast([128, 128]),
                                identity=ident[:, :])
                            nc.scalar.copy(out=gb_sb[e][:, tb * 128:(tb + 1) * 128], in_=t_ps)
                        t_ps = bps.tile([128, 128], f32, tag="bc")
                        nc.tensor.transpose(
                            out=t_ps[:, :],
                            in_=rs[:, tb:tb + 1].to_broadcast([128, 128]),
                            identity=ident[:, :])
                        nc.scalar.copy(out=rb_sb[:, tb * 128:(tb + 1) * 128], in_=t_ps)

                # ---- experts (transposed layout) + gated combine ----
                enhT = pA.tile([128, DC, T], f32)
                with tc.tile_pool(name="pw", bufs=2) as pw, \
                     tc.tile_pool(name="eps", bufs=2, space="PSUM") as eps:
                    for co in range(DC):
                        wt = {}
                        for e in range(3):
                            wt[(e, "h")] = pw.tile([128, DC, 128], bf16, tag=f"w{e}h", name=f"w{e}h")
                            nc.sync.dma_start(
                                out=wt[(e, "h")],
                                in_=Wh[e][:, co * 128:(co + 1) * 128].rearrange("(c p) n -> p c n", p=128))
                            wt[(e, "l")] = pw.tile([128, DC, 128], bf16, tag=f"w{e}l", name=f"w{e}l")
                            nc.sync.dma_start(
                                out=wt[(e, "l")],
                                in_=Wl[e][:, co * 128:(co + 1) * 128].rearrange("(c p) n -> p c n", p=128))
                        for half in range(2):
                            hs = slice(half * 512, (half + 1) * 512)
                            e_ps = []
                            for e in range(3):
                                ps_ = eps.tile([128, 512], f32, tag=f"e{e}")
                                first = True
                                for (xs, wk) in ((xh_t, "h"), (xh_t, "l"), (xl_t, "h")):
                                    for c in range(DC):
                                        last = (not with_bias) and xs is xl_t and c == DC - 1
                                        nc.tensor.matmul(
                                            out=ps_[:, :], lhsT=wt[(e, wk)][:, c, :],
                                            rhs=xs[:, c, hs], start=first, stop=last)
                                        first = False
                                if with_bias:
                                    nc.tensor.matmul(
                                        out=ps_[:, :], lhsT=bexph_t[e:e + 1, co * 128:(co + 1) * 128],
                                        rhs=xon[0:1, hs], start=False, stop=False)
                                    nc.tensor.matmul(
                                        out=ps_[:, :], lhsT=bexpl_t[e:e + 1, co * 128:(co + 1) * 128],
                                        rhs=xon[0:1, hs], start=False, stop=True)
                                e_ps.append(ps_)
                            uacc = pA.tile([128, 512], f32, tag="uacc")
                            vtmp = pA.tile([128, 512], f32, tag="vtmp")
                            nc.vector.tensor_tensor(out=uacc, in0=e_ps[0], in1=gb_sb[0][:, hs], op=OP.mult)
                            nc.vector.tensor_tensor(out=vtmp, in0=e_ps[1], in1=gb_sb[1][:, hs], op=OP.mult)
                            nc.vector.tensor_tensor(out=uacc, in0=uacc, in1=vtmp, op=OP.add)
                            nc.vector.tensor_tensor(out=vtmp, in0=e_ps[2], in1=gb_sb[2][:, hs], op=OP.mult)
                            nc.vector.tensor_tensor(out=uacc, in0=uacc, in1=vtmp, op=OP.add)
                            nc.vector.tensor_tensor(out=uacc, in0=uacc, in1=rb_sb[:, hs], op=OP.mult)
                            nc.vector.tensor_scalar(out=enhT[:, co, hs], in0=uacc,
                                                    scalar1=vbar_t[:, co:co + 1], scalar2=None,
                                                    op0=OP.add)

                # ---- split enhancedT into bf16 hi/lo ----
                for c in range(DC):
                    nc.vector.tensor_copy(out=EhT[:, c, :], in_=enhT[:, c, :])
                    nc.vector.tensor_tensor(out=ElT[:, c, :], in0=enhT[:, c, :],
                                            in1=EhT[:, c, :], op=OP.subtract)

            # =========================== phase B ===========================
            with tc.tile_pool(name="pB", bufs=1) as pB, \
                 tc.tile_pool(name="pBs", bufs=3) as pBs, \
                 tc.tile_pool(name="lps", bufs=4, space="PSUM") as lps, \
                 tc.tile_pool(name="qps", bufs=2, space="PSUM") as qps:
                expbuf = pB.tile([128, TB_PER_SW, KCB], f16)
                logit_dmas = {tb: [] for tb in range(NTB)}

                def sweep(sw):
                    tbs = range(sw * TB_PER_SW, (sw + 1) * TB_PER_SW)
                    for nb in range(NNB):
                        ns = slice(nb * 512, (nb + 1) * 512)
                        wlh_t = pBs.tile([128, DC, 512], bf16, tag="wlh")
                        nc.sync.dma_start(out=wlh_t, in_=WLh[:, ns].rearrange("(c p) n -> p c n", p=128))
                        wll_t = pBs.tile([128, DC, 512], bf16, tag="wll")
                        nc.sync.dma_start(out=wll_t, in_=WLl[:, ns].rearrange("(c p) n -> p c n", p=128))
                        if with_bias:
                            blh_t = pBs.tile([1, 512], bf16, tag="blh")
                            nc.sync.dma_start(out=blh_t, in_=blh[:, ns])
                            bll_t = pBs.tile([1, 512], bf16, tag="bll")
                            nc.sync.dma_start(out=bll_t, in_=bll[:, ns])
                        for tb in tbs:
                            ts_ = slice(tb * 128, (tb + 1) * 128)
                            lp = lps.tile([128, 512], f32, tag="lp")
                            first = True
                            for (es, ws) in ((EhT, wlh_t), (EhT, wll_t), (ElT, wlh_t)):
                                for c in range(DC):
                                    last = (not with_bias) and es is ElT and c == DC - 1
                                    nc.tensor.matmul(out=lp[:, :], lhsT=es[:, c, ts_],
                                                     rhs=ws[:, c, :], start=first, stop=last)
                                    first = False
                            if with_bias:
                                nc.tensor.matmul(out=lp[:, :], lhsT=ones_h[0:1, ts_],
                                                 rhs=blh_t[0:1, :], start=False, stop=False)
                                nc.tensor.matmul(out=lp[:, :], lhsT=ones_h[0:1, ts_],
                                                 rhs=bll_t[0:1, :], start=False, stop=True)
                            stg = pBs.tile([128, 512], f32, tag="lstg")
                            nc.scalar.copy(out=stg, in_=lp)
                            dmi = nc.sync.dma_start(out=logits_o[ts_, ns], in_=stg)
                            logit_dmas[tb].append(dmi)
                            nc.scalar.activation(
                                out=expbuf[:, tb - sw * TB_PER_SW, ns], in_=lp, func=AF.Exp,
                                bias=0.0, scale=1.0,
                                accum_out=spart[:, tb, nb:nb + 1])
                            nc.vector.tensor_reduce(out=mpart[:, tb, nb:nb + 1], in_=lp,
                                                    axis=AX, op=OP.max)

                def finalize(tb, sw):
                    ts_ = slice(tb * 128, (tb + 1) * 128)
                    lt = tb - sw * TB_PER_SW
                    # softmax scale r = 1/S (newton)
                    s1 = pBs.tile([128, 1], f32, tag="s1")
                    nc.vector.tensor_reduce(out=s1, in_=spart[:, tb, :], axis=AX, op=OP.add)
                    r0 = pBs.tile([128, 1], f32, tag="r0")
                    nc.vector.reciprocal(out=r0, in_=s1)
                    tn = pBs.tile([128, 1], f32, tag="tn")
                    nc.vector.tensor_tensor(out=tn, in0=s1, in1=r0, op=OP.mult)
                    nc.vector.tensor_scalar(out=tn, in0=tn, scalar1=2.0, scalar2=-1.0,
                                            op0=OP.subtract, op1=OP.mult)
                    rr = pBs.tile([128, 1], f32, tag="rr")
                    nc.vector.tensor_tensor(out=rr, in0=r0, in1=tn, op=OP.mult)
                    # argmax: row max + winning block
                    mx = pBs.tile([128, 1], f32, tag="mx")
                    nc.vector.tensor_reduce(out=mx, in_=mpart[:, tb, :], axis=AX, op=OP.max)
                    bsl = pBs.tile([128, NNB], f32, tag="bsl")
                    nc.vector.scalar_tensor_tensor(out=bsl, in0=mpart[:, tb, :], scalar=mx,
                                                   in1=iota16, op0=OP.is_equal, op1=OP.mult)
                    bv = pBs.tile([128, 1], f32, tag="bv")
                    nc.vector.tensor_reduce(out=bv, in_=bsl, axis=AX, op=OP.max)
                    bstar = pBs.tile([128, 1], f32, tag="bstar")
                    nc.vector.tensor_scalar(out=bstar, in0=bv, scalar1=float(NNB - 1),
                                            scalar2=-1.0, op0=OP.subtract, op1=OP.mult)
                    # gather the winning 512-block of this tb's logits rows
                    rrow = pBs.tile([128, 1], f32, tag="rrow")
                    nc.vector.scalar_tensor_tensor(out=rrow, in0=tokio[:, tb:tb + 1],
                                                   scalar=float(NNB), in1=bstar,
                                                   op0=OP.mult, op1=OP.add)
                    ri = pBs.tile([128, 1], i32, tag="ri")
                    nc.vector.tensor_copy(out=ri, in_=rrow)
                    gl = pBs.tile([128, 512], f32, tag="gl")
                    gth = nc.gpsimd.indirect_dma_start(
                        out=gl[:, :], out_offset=None, in_=logits_rows,
                        in_offset=bass.IndirectOffsetOnAxis(ap=ri[:, :1], axis=0))
                    for dmi in logit_dmas[tb]:
                        add_dep_helper(gth.ins, dmi.ins, reason="gather logits after writeback")
                    psl = pBs.tile([128, 512], f32, tag="psl")
                    nc.vector.scalar_tensor_tensor(out=psl, in0=gl, scalar=mx, in1=iota512,
                                                   op0=OP.is_equal, op1=OP.mult)
                    pv_ = pBs.tile([128, 1], f32, tag="pv_")
                    nc.vector.tensor_reduce(out=pv_, in_=psl, axis=AX, op=OP.max)
                    pos = pBs.tile([128, 1], f32, tag="pos")
                    nc.vector.tensor_scalar(out=pos, in0=pv_, scalar1=511.0, scalar2=-1.0,
                                            op0=OP.subtract, op1=OP.mult)
                    idxf = pBs.tile([128, 1], f32, tag="idxf")
                    nc.vector.scalar_tensor_tensor(out=idxf, in0=bstar, scalar=512.0,
                                                   in1=pos, op0=OP.mult, op1=OP.add)
                    nc.sync.dma_start(out=idx_o[tb:tb + 1, :], in_=idxf[:, 0:1])
                    idxi = pBs.tile([128, 1], i32, tag="idxi")
                    nc.vector.tensor_copy(out=idxi, in_=idxf)
                    # quantized = codebook[idx]
                    qg = pBs.tile([128, D], f32, tag="qg")
                    nc.gpsimd.indirect_dma_start(
                        out=qg[:, :], out_offset=None, in_=cb[:, :],
                        in_offset=bass.IndirectOffsetOnAxis(ap=idxi[:, :1], axis=0))
                    nc.sync.dma_start(out=quant_o[ts_, :], in_=qg)
                    # soft = expbuf * r
                    for ch in range(4):
                        cs = slice(ch * 2048, (ch + 1) * 2048)
                        sst = pBs.tile([128, 2048], f32, tag="sst")
                        nc.vector.tensor_scalar(out=sst, in0=expbuf[:, lt, cs],
                                                scalar1=rr, scalar2=None, op0=OP.mult)
                        nc.sync.dma_start(out=soft_o[ts_, cs], in_=sst)
                    # vq loss partials: sum_d (q - enh)^2 in transposed layout
                    for c in range(DC):
                        qt_ps = qps.tile([128, 128], f32, tag="qt")
                        nc.tensor.matmul(out=qt_ps[:, :], lhsT=qg[:, c * 128:(c + 1) * 128],
                                         rhs=ident[:, :], start=True, stop=True)
                        df = pBs.tile([128, 128], f32, tag="df")
                        nc.vector.tensor_tensor(out=df, in0=qt_ps, in1=EhT[:, c, ts_], op=OP.subtract)
                        nc.vector.tensor_tensor(out=df, in0=df, in1=ElT[:, c, ts_], op=OP.subtract)
                        sqs = pBs.tile([128, 128], f32, tag="sqs")
                        nc.scalar.activation(out=sqs, in_=df, func=AF.Square,
                                             bias=0.0, scale=1.0,
                                             accum_out=lossp[:, tb * DC + c:tb * DC + c + 1])

                for sw in range(NSW):
                    sweep(sw)
                    for tb in range(sw * TB_PER_SW, (sw + 1) * TB_PER_SW):
                        finalize(tb, sw)

                lsum = pBs.tile([128, 1], f32, tag="lsum")
                nc.vector.tensor_reduce(out=lsum, in_=lossp, axis=AX, op=OP.add)
                nc.sync.dma_start(out=loss_o[:, :], in_=lsum)

    nc.finalize()
    return nc


# ----------------------------- host side -----------------------------

_NC_CACHE = {}


def _get_nc(with_bias: bool):
    if with_bias not in _NC_CACHE:
        _NC_CACHE[with_bias] = build_nc(with_bias)
    return _NC_CACHE[with_bias]


def _split(a):
    a = np.ascontiguousarray(a, dtype=np.float32)
    h = a.astype(_BF)
    l = (a - h.astype(np.float32)).astype(_BF)
    return h, l


def _prep(inputs):
    x = np.asarray(inputs["x"], np.float32)
    with_bias = any(
        np.any(np.asarray(inputs[k])) for k in ("b_low", "b_mid", "b_high", "b_gate", "b_logits"))

    W = {}
    for e, nm in enumerate(("W_low", "W_mid", "W_high")):
        h, l = _split(np.asarray(inputs[nm]))
        W[f"W{e}h"] = h
        W[f"W{e}l"] = l
    W["Wgh"], W["Wgl"] = _split(np.asarray(inputs["W_gate"]))
    W["WLh"], W["WLl"] = _split(np.asarray(inputs["W_logits"]))
    vb = np.asarray(inputs["vehicle_emb"], np.float32).mean(axis=0)
    W["vbar"] = np.ascontiguousarray(vb.reshape(DC, 128).T, np.float32)
    W["cb"] = np.ascontiguousarray(np.asarray(inputs["codebook"], np.float32))
    if with_bias:
        bh, bl = _split(np.stack([np.asarray(inputs[k], np.float32)
                                  for k in ("b_low", "b_mid", "b_high")]))
        W["bexph"], W["bexpl"] = bh, bl
        gh, gl = _split(np.asarray(inputs["b_gate"], np.float32)[None, :])
        W["bgh"], W["bgl"] = gh, gl
        lh, ll = _split(np.asarray(inputs["b_logits"], np.float32)[None, :])
        W["blh"], W["bll"] = lh, ll

    in_maps = []
    for c in range(B):
        xT = np.ascontiguousarray(x[c].T)
        xh, xl = _split(xT)
        m = {"xTh": xh, "xTl": xl}
        m.update(W)
        in_maps.append(m)
    return in_maps, with_bias


def _run(inputs, trace=False):
    from concourse.bass_utils import run_bass_kernel_spmd
    in_maps, with_bias = _prep(inputs)
    nc = _get_nc(with_bias)
    res = run_bass_kernel_spmd(nc, in_maps, list(range(B)), trace=trace)
    return res


def _assemble(res):
    quant = np.stack([res.results[c]["quant_o"] for c in range(B)])
    logits = np.stack([res.results[c]["logits_o"] for c in range(B)])
    soft = np.stack([res.results[c]["soft_o"] for c in range(B)])
    idx = np.stack([np.rint(res.results[c]["idx_o"].reshape(T)).astype(np.int32)
                    for c in range(B)])
    tot = sum(float(res.results[c]["loss_o"].sum(dtype=np.float64)) for c in range(B))
    vq_loss = np.float32(tot / (B * T * D))
    return quant, idx, vq_loss, soft, logits


def kernel(**inputs):
    res = _run(inputs, trace=False)
    return _assemble(res)


def model_time_ns(with_bias=False):
    """Cost-model execution time (no NTFF profiling available under this
    axon container, so CoreSim's instruction cost model is the ns source)."""
    import concourse.bass_interp as bass_interp
    nc = _get_nc(with_bias)
    sim = bass_interp.CoreSim(nc, no_exec=True, publish_trace=False)
    sim.simulate()
    return int(sim.time)


def kernel_profiled(**inputs):
    res = _run(inputs, trace=False)
    out = _assemble(res)
    return out, model_time_ns()


# revision 18
# speedup vs baseline: 1.0662x; 1.0662x over previous
"""TRN2 Bass kernel for the BEATs-style VQ tokenizer (vq_codebook problem).

Data-parallel over batch B=8 across 8 NeuronCores. Each core processes its
1024-token slice end to end:

  phase A: gate (token layout, bf16x3 matmuls + DVE polynomial exp softmax),
           three expert matmuls in transposed [d, token] layout (bf16x3),
           gated combine via PE broadcast-transposes + DVE, vehicle-mean add,
           then split enhancedT into bf16 hi/lo for the logits matmuls.
  phase B: logits = enhancedT.T @ W_logits as bf16x3 (hi*hi + hi*lo + lo*hi),
           tiled [128 tok, 512 cb] psum tiles; per tile: ScalarE copy to SBUF
           (-> HBM logits out), ScalarE exp (fp16, accum row-sums), VectorE
           block-max. Two sweeps of W_logits (4 token blocks each) keep the
           fp16 exp buffer within SBUF.
  finalize per token block: softmax scale (Newton-refined reciprocal),
           exact argmax (block max -> winning block gather from HBM ->
           position via iota/is_equal), codebook gather, vq-loss partials.

Precision: all matmuls are bf16 hi/lo split x3 (exact products, f32 psum
accumulation) giving ~4e-5 absmax logit error vs the f32 reference, far under
the 1.8e-5..~0.1 top-2 logit gaps -> argmax matches the reference exactly.
quantized == codebook[idx] holds bitwise in the reference (verified).
"""

import numpy as np
import ml_dtypes

import concourse.bass as bass
import concourse.bacc as bacc
import concourse.mybir as mybir
import concourse.tile as tile
from concourse.tile_rust import add_dep_helper

B, T, D, KCB = 8, 1024, 768, 8192
NTB = 8          # token blocks of 128 per core
NNB = 16         # codebook-dim blocks of 512
DC = 6           # d chunks of 128
SWEEP_TBS = [list(range(0, 5)), list(range(5, 8))]  # W_logits sweeps (5/3 split)

f32 = mybir.dt.float32
f16 = mybir.dt.float16
bf16 = mybir.dt.bfloat16
i32 = mybir.dt.int32
AX = mybir.AxisListType.X
OP = mybir.AluOpType
AF = mybir.ActivationFunctionType

_BF = ml_dtypes.bfloat16


def _exp_poly_coeffs():
    """Power-basis coeffs (in u = x/4 + 2, u in [-2,2]) approximating
    e^(u-2); gate exp is then poly(u)^4 = e^x for x in [-16, 0]."""
    u = np.linspace(-2, 2, 20001)
    cheb = np.polynomial.chebyshev.Chebyshev.fit(u, np.exp(u - 2.0), deg=15)
    c = cheb.convert(kind=np.polynomial.Polynomial).coef
    rel = np.abs(np.polyval(c[::-1], u) / np.exp(u - 2.0) - 1.0).max()
    assert rel < 1e-7, rel
    return [float(v) for v in c]  # c[k] multiplies u^k

_EXP_C = _exp_poly_coeffs()


def build_nc(with_bias: bool):
    nc = bacc.Bacc(None)

    # ---- inputs (per core) ----
    xTh = nc.dram_tensor("xTh", [D, T], bf16, kind="ExternalInput")
    xTl = nc.dram_tensor("xTl", [D, T], bf16, kind="ExternalInput")
    Wh = [nc.dram_tensor(f"W{e}h", [D, D], bf16, kind="ExternalInput") for e in range(3)]
    Wl = [nc.dram_tensor(f"W{e}l", [D, D], bf16, kind="ExternalInput") for e in range(3)]
    Wgh = nc.dram_tensor("Wgh", [D, 3], bf16, kind="ExternalInput")
    Wgl = nc.dram_tensor("Wgl", [D, 3], bf16, kind="ExternalInput")
    WLh = nc.dram_tensor("WLh", [D, KCB], bf16, kind="ExternalInput")
    WLl = nc.dram_tensor("WLl", [D, KCB], bf16, kind="ExternalInput")
    vbar = nc.dram_tensor("vbar", [128, DC], f32, kind="ExternalInput")
    cb = nc.dram_tensor("cb", [KCB, D], f32, kind="ExternalInput")
    if with_bias:
        bexph = nc.dram_tensor("bexph", [3, D], bf16, kind="ExternalInput")
        bexpl = nc.dram_tensor("bexpl", [3, D], bf16, kind="ExternalInput")
        bgh = nc.dram_tensor("bgh", [1, 3], bf16, kind="ExternalInput")
        bgl = nc.dram_tensor("bgl", [1, 3], bf16, kind="ExternalInput")
        blh = nc.dram_tensor("blh", [1, KCB], bf16, kind="ExternalInput")
        bll = nc.dram_tensor("bll", [1, KCB], bf16, kind="ExternalInput")

    # ---- outputs (per core) ----
    logits_o = nc.dram_tensor("logits_o", [T, KCB], f32, kind="ExternalOutput")
    soft_o = nc.dram_tensor("soft_o", [T, KCB], f32, kind="ExternalOutput")
    quant_o = nc.dram_tensor("quant_o", [T, D], f32, kind="ExternalOutput")
    idx_o = nc.dram_tensor("idx_o", [NTB, 128], f32, kind="ExternalOutput")
    loss_o = nc.dram_tensor("loss_o", [128, 1], f32, kind="ExternalOutput")

    logits_rows = logits_o[:, :].rearrange("t (b c) -> (t b) c", c=512)

    with tile.TileContext(nc) as tc:
        with tc.tile_pool(name="pers", bufs=1) as pers:
            # persistent tiles
            EhT = pers.tile([128, DC, T], bf16)
            ElT = pers.tile([128, DC, T], bf16)
            spart = pers.tile([128, NTB, NNB], f32)
            mpart = pers.tile([128, NTB, NNB], f32)
            lossp = pers.tile([128, NTB * DC], f32)
            iota512 = pers.tile([128, 512], f32)
            iota16 = pers.tile([128, NNB], f32)
            ident = pers.tile([128, 128], f32)
            vbar_t = pers.tile([128, DC], f32)
            tokio = pers.tile([128, NTB], f32)
            ones_h = pers.tile([1, T], bf16)

            # constants
            it512 = pers.tile([128, 512], i32)
            nc.gpsimd.iota(it512, pattern=[[-1, 512]], base=511, channel_multiplier=0)
            nc.vector.tensor_copy(out=iota512, in_=it512)
            it16 = pers.tile([128, NNB], i32)
            nc.gpsimd.iota(it16, pattern=[[-1, NNB]], base=NNB - 1, channel_multiplier=0)
            nc.vector.tensor_copy(out=iota16, in_=it16)
            itok = pers.tile([128, NTB], i32)
            nc.gpsimd.iota(itok, pattern=[[128, NTB]], base=0, channel_multiplier=1)
            nc.vector.tensor_copy(out=tokio, in_=itok)
            from concourse.masks import make_identity
            make_identity(nc, ident)
            nc.sync.dma_start(out=vbar_t, in_=vbar[:, :])
            nc.vector.memset(ones_h, 1.0)

            # =========================== phase A ===========================
            pWL_cm = tc.tile_pool(name="pWL", bufs=2)
            pWL = pWL_cm.__enter__()
            with tc.tile_pool(name="pA", bufs=1) as pA:
                xh_t = pA.tile([128, DC, T], bf16)
                xl_t = pA.tile([128, DC, T], bf16)
                for c in range(DC):
                    nc.sync.dma_start(out=xh_t[:, c, :], in_=xTh[c * 128:(c + 1) * 128, :])
                    nc.scalar.dma_start(out=xl_t[:, c, :], in_=xTl[c * 128:(c + 1) * 128, :])
                wgh_t = pA.tile([128, DC, 3], bf16)
                nc.sync.dma_start(out=wgh_t, in_=Wgh[:, :].rearrange("(c p) n -> p c n", p=128))
                wgl_t = pA.tile([128, DC, 3], bf16)
                nc.sync.dma_start(out=wgl_t, in_=Wgl[:, :].rearrange("(c p) n -> p c n", p=128))
                if with_bias:
                    bgh_t = pA.tile([1, 3], bf16)
                    nc.sync.dma_start(out=bgh_t, in_=bgh[:, :])
                    bgl_t = pA.tile([1, 3], bf16)
                    nc.sync.dma_start(out=bgl_t, in_=bgl[:, :])
                    xon = pA.tile([1, T], bf16)
                    nc.vector.memset(xon, 1.0)
                    bexph_t = pA.tile([3, D], bf16)
                    nc.sync.dma_start(out=bexph_t, in_=bexph[:, :])
                    bexpl_t = pA.tile([3, D], bf16)
                    nc.sync.dma_start(out=bexpl_t, in_=bexpl[:, :])

                # ---- gate logits, token layout [128 tok, 3] per tb ----
                glog = pA.tile([128, NTB, 3], f32)
                with tc.tile_pool(name="gps", bufs=2, space="PSUM") as gps:
                    for tb in range(NTB):
                        g_ps = gps.tile([128, 3], f32, tag="g")
                        first = True
                        for si, (xs, ws) in enumerate(((xh_t, wgh_t), (xh_t, wgl_t), (xl_t, wgh_t))):
                            for c in range(DC):
                                last = (not with_bias) and si == 2 and c == DC - 1
                                nc.tensor.matmul(
                                    out=g_ps[:, :],
                                    lhsT=xs[:, c, tb * 128:(tb + 1) * 128],
                                    rhs=ws[:, c, :],
                                    start=first, stop=last)
                                first = False
                        if with_bias:
                            nc.tensor.matmul(out=g_ps[:, :], lhsT=xon[0:1, tb * 128:(tb + 1) * 128],
                                             rhs=bgh_t[0:1, :], start=False, stop=False)
                            nc.tensor.matmul(out=g_ps[:, :], lhsT=xon[0:1, tb * 128:(tb + 1) * 128],
                                             rhs=bgl_t[0:1, :], start=False, stop=True)
                        nc.scalar.copy(out=glog[:, tb, :], in_=g_ps[:, :])

                # ---- gate softmax: shift by max, DVE polynomial exp ----
                cent = pA.tile([128, NTB, 3], f32)
                for tb in range(NTB):
                    nm = pA.tile([128, 1], f32, tag="gnm")
                    nc.vector.tensor_reduce(out=nm, in_=glog[:, tb, :], axis=AX,
                                            op=OP.max, negate=True)
                    nc.vector.tensor_scalar(out=cent[:, tb, :], in0=glog[:, tb, :],
                                            scalar1=nm, scalar2=None, op0=OP.add)
                flat = cent[:, :, :].rearrange("p a b -> p (a b)")
                u = pA.tile([128, NTB * 3], f32)
                # u = max(x, -16) * 0.25 + 2
                nc.vector.tensor_scalar(out=u, in0=flat, scalar1=-16.0, scalar2=0.25,
                                        op0=OP.max, op1=OP.mult)
                nc.vector.tensor_scalar(out=u, in0=u, scalar1=2.0, scalar2=None, op0=OP.add)
                pv = pA.tile([128, NTB * 3], f32)
                nc.vector.memset(pv, 0.0)
                for k in range(15, 0, -1):
                    nc.vector.scalar_tensor_tensor(out=pv, in0=pv, scalar=float(_EXP_C[k]),
                                                   in1=u, op0=OP.add, op1=OP.mult)
                nc.vector.tensor_scalar(out=pv, in0=pv, scalar1=float(_EXP_C[0]),
                                        scalar2=None, op0=OP.add)
                nc.vector.tensor_tensor(out=pv, in0=pv, in1=pv, op=OP.mult)
                nc.vector.tensor_tensor(out=pv, in0=pv, in1=pv, op=OP.mult)
                gexp = pv.rearrange("p (a b) -> p a b", b=3)

                # row sums + newton reciprocal -> rs [128, NTB]
                gs = pA.tile([128, NTB], f32)
                nc.vector.tensor_reduce(out=gs, in_=gexp, axis=AX, op=OP.add)
                rs0 = pA.tile([128, NTB], f32)
                nc.vector.reciprocal(out=rs0, in_=gs)
                tnw = pA.tile([128, NTB], f32)
                nc.vector.tensor_tensor(out=tnw, in0=gs, in1=rs0, op=OP.mult)
                nc.vector.tensor_scalar(out=tnw, in0=tnw, scalar1=2.0, scalar2=-1.0,
                                        op0=OP.subtract, op1=OP.mult)
                rs = pA.tile([128, NTB], f32)
                nc.vector.tensor_tensor(out=rs, in0=rs0, in1=tnw, op=OP.mult)

                # ---- broadcast g0/g1/g2 and 1/s across partitions ----
                gb_sb = [pA.tile([128, T], f32, tag=f"gb{e}", name=f"gb{e}") for e in range(3)]
                rb_sb = pA.tile([128, T], f32, tag="rb")
                with tc.tile_pool(name="bps", bufs=3, space="PSUM") as bps:
                    for tb in range(NTB):
                        for e in range(3):
                            t_ps = bps.tile([128, 128], f32, tag="bc")
                            nc.tensor.transpose(
                                out=t_ps[:, :],
                                in_=gexp[:, tb, e:e + 1].to_broadcast([128, 128]),
                                identity=ident[:, :])
                            nc.scalar.copy(out=gb_sb[e][:, tb * 128:(tb + 1) * 128], in_=t_ps)
                        t_ps = bps.tile([128, 128], f32, tag="bc")
                        nc.tensor.transpose(
                            out=t_ps[:, :],
                            in_=rs[:, tb:tb + 1].to_broadcast([128, 128]),
                            identity=ident[:, :])
                        nc.scalar.copy(out=rb_sb[:, tb * 128:(tb + 1) * 128], in_=t_ps)

                # ---- experts (transposed layout) + gated combine ----
                enhT = pA.tile([128, DC, T], f32)
                with tc.tile_pool(name="pw", bufs=2) as pw, \
                     tc.tile_pool(name="eps", bufs=2, space="PSUM") as eps:
                    for co in range(DC):
                        wt = {}
                        for e in range(3):
                            wt[(e, "h")] = pw.tile([128, DC, 128], bf16, tag=f"w{e}h", name=f"w{e}h")
                            nc.sync.dma_start(
                                out=wt[(e, "h")],
                                in_=Wh[e][:, co * 128:(co + 1) * 128].rearrange("(c p) n -> p c n", p=128))
                            wt[(e, "l")] = pw.tile([128, DC, 128], bf16, tag=f"w{e}l", name=f"w{e}l")
                            nc.scalar.dma_start(
                                out=wt[(e, "l")],
                                in_=Wl[e][:, co * 128:(co + 1) * 128].rearrange("(c p) n -> p c n", p=128))
                        for half in range(2):
                            hs = slice(half * 512, (half + 1) * 512)
                            e_ps = []
                            for e in range(3):
                                ps_ = eps.tile([128, 512], f32, tag=f"e{e}")
                                first = True
                                for (xs, wk) in ((xh_t, "h"), (xh_t, "l"), (xl_t, "h")):
                                    for c in range(DC):
                                        last = (not with_bias) and xs is xl_t and c == DC - 1
                                        nc.tensor.matmul(
                                            out=ps_[:, :], lhsT=wt[(e, wk)][:, c, :],
                                            rhs=xs[:, c, hs], start=first, stop=last)
                                        first = False
                                if with_bias:
                                    nc.tensor.matmul(
                                        out=ps_[:, :], lhsT=bexph_t[e:e + 1, co * 128:(co + 1) * 128],
                                        rhs=xon[0:1, hs], start=False, stop=False)
                                    nc.tensor.matmul(
                                        out=ps_[:, :], lhsT=bexpl_t[e:e + 1, co * 128:(co + 1) * 128],
                                        rhs=xon[0:1, hs], start=False, stop=True)
                                e_ps.append(ps_)
                            uacc = pA.tile([128, 512], f32, tag="uacc")
                            vtmp = pA.tile([128, 512], f32, tag="vtmp")
                            nc.vector.tensor_tensor(out=uacc, in0=e_ps[0], in1=gb_sb[0][:, hs], op=OP.mult)
                            nc.vector.tensor_tensor(out=vtmp, in0=e_ps[1], in1=gb_sb[1][:, hs], op=OP.mult)
                            nc.vector.tensor_tensor(out=uacc, in0=uacc, in1=vtmp, op=OP.add)
                            nc.vector.tensor_tensor(out=vtmp, in0=e_ps[2], in1=gb_sb[2][:, hs], op=OP.mult)
                            nc.vector.tensor_tensor(out=uacc, in0=uacc, in1=vtmp, op=OP.add)
                            nc.vector.tensor_tensor(out=uacc, in0=uacc, in1=rb_sb[:, hs], op=OP.mult)
                            nc.vector.tensor_scalar(out=enhT[:, co, hs], in0=uacc,
                                                    scalar1=vbar_t[:, co:co + 1], scalar2=None,
                                                    op0=OP.add)
                        # split this chunk of enhancedT into bf16 hi/lo right
                        # away so phase B can start as soon as all chunks land
                        nc.vector.tensor_copy(out=EhT[:, co, :], in_=enhT[:, co, :])
                        nc.vector.tensor_tensor(out=ElT[:, co, :], in0=enhT[:, co, :],
                                                in1=EhT[:, co, :], op=OP.subtract)

            # =========================== phase B ===========================
            with tc.tile_pool(name="pB", bufs=1) as pB, \
                 tc.tile_pool(name="pBs", bufs=3) as pBs, \
                 tc.tile_pool(name="lps", bufs=6, space="PSUM") as lps, \
                 tc.tile_pool(name="qps", bufs=2, space="PSUM") as qps:
                NSLOT = 6  # ring of exp slots: sweep 1 starts on fresh slots
                expbuf = pB.tile([128, NSLOT, KCB], f16)
                logit_dmas = {tb: [] for tb in range(NTB)}

                def sweep(sw, fc_sched=None):
                    tbs = SWEEP_TBS[sw]
                    for nb in range(NNB):
                        if fc_sched and nb in fc_sched:
                            for _tb in fc_sched[nb]:
                                finalize(_tb)
                        ns = slice(nb * 512, (nb + 1) * 512)
                        wlh_t = pWL.tile([128, DC, 512], bf16, tag="wlh")
                        nc.sync.dma_start(out=wlh_t, in_=WLh[:, ns].rearrange("(c p) n -> p c n", p=128))
                        wll_t = pWL.tile([128, DC, 512], bf16, tag="wll")
                        nc.scalar.dma_start(out=wll_t, in_=WLl[:, ns].rearrange("(c p) n -> p c n", p=128))
                        if with_bias:
                            blh_t = pBs.tile([1, 512], bf16, tag="blh")
                            nc.sync.dma_start(out=blh_t, in_=blh[:, ns])
                            bll_t = pBs.tile([1, 512], bf16, tag="bll")
                            nc.sync.dma_start(out=bll_t, in_=bll[:, ns])
                        for tb in tbs:
                            ts_ = slice(tb * 128, (tb + 1) * 128)
                            lp = lps.tile([128, 512], f32, tag="lp")
                            first = True
                            for (es, ws) in ((EhT, wlh_t), (EhT, wll_t), (ElT, wlh_t)):
                                for c in range(DC):
                                    last = (not with_bias) and es is ElT and c == DC - 1
                                    nc.tensor.matmul(out=lp[:, :], lhsT=es[:, c, ts_],
                                                     rhs=ws[:, c, :], start=first, stop=last)
                                    first = False
                            if with_bias:
                                nc.tensor.matmul(out=lp[:, :], lhsT=ones_h[0:1, ts_],
                                                 rhs=blh_t[0:1, :], start=False, stop=False)
                                nc.tensor.matmul(out=lp[:, :], lhsT=ones_h[0:1, ts_],
                                                 rhs=bll_t[0:1, :], start=False, stop=True)
                            stg = pBs.tile([128, 512], f32, tag="lstg")
                            nc.scalar.copy(out=stg, in_=lp)
                            dmi = nc.scalar.dma_start(out=logits_o[ts_, ns], in_=stg)
                            logit_dmas[tb].append(dmi)
                            nc.scalar.activation(
                                out=expbuf[:, tb % NSLOT, ns], in_=lp, func=AF.Exp,
                                bias=0.0, scale=1.0,
                                accum_out=spart[:, tb, nb:nb + 1])
                            nc.vector.tensor_reduce(out=mpart[:, tb, nb:nb + 1], in_=lp,
                                                    axis=AX, op=OP.max)

                def finalize(tb):
                    ts_ = slice(tb * 128, (tb + 1) * 128)
                    lt = tb % NSLOT
                    # softmax scale r = 1/S (newton)
                    s1 = pBs.tile([128, 1], f32, tag="s1")
                    nc.vector.tensor_reduce(out=s1, in_=spart[:, tb, :], axis=AX, op=OP.add)
                    r0 = pBs.tile([128, 1], f32, tag="r0")
                    nc.vector.reciprocal(out=r0, in_=s1)
                    tn = pBs.tile([128, 1], f32, tag="tn")
                    nc.vector.tensor_tensor(out=tn, in0=s1, in1=r0, op=OP.mult)
                    nc.vector.tensor_scalar(out=tn, in0=tn, scalar1=2.0, scalar2=-1.0,
                                            op0=OP.subtract, op1=OP.mult)
                    rr = pBs.tile([128, 1], f32, tag="rr")
                    nc.vector.tensor_tensor(out=rr, in0=r0, in1=tn, op=OP.mult)
                    # soft = expbuf * r first: frees this tb's exp slot ASAP
                    for ch in range(4):
                        cs = slice(ch * 2048, (ch + 1) * 2048)
                        sst = pBs.tile([128, 2048], f32, tag="sst", name="sst", bufs=2)
                        nc.vector.tensor_scalar(out=sst, in0=expbuf[:, lt, cs],
                                                scalar1=rr, scalar2=None, op0=OP.mult)
                        nc.sync.dma_start(out=soft_o[ts_, cs], in_=sst)
                    # argmax: row max + winning block
                    mx = pBs.tile([128, 1], f32, tag="mx")
                    nc.vector.tensor_reduce(out=mx, in_=mpart[:, tb, :], axis=AX, op=OP.max)
                    bsl = pBs.tile([128, NNB], f32, tag="bsl")
                    nc.vector.scalar_tensor_tensor(out=bsl, in0=mpart[:, tb, :], scalar=mx,
                                                   in1=iota16, op0=OP.is_equal, op1=OP.mult)
                    bv = pBs.tile([128, 1], f32, tag="bv")
                    nc.vector.tensor_reduce(out=bv, in_=bsl, axis=AX, op=OP.max)
                    bstar = pBs.tile([128, 1], f32, tag="bstar")
                    nc.vector.tensor_scalar(out=bstar, in0=bv, scalar1=float(NNB - 1),
                                            scalar2=-1.0, op0=OP.subtract, op1=OP.mult)
                    # gather the winning 512-block of this tb's logits rows
                    rrow = pBs.tile([128, 1], f32, tag="rrow")
                    nc.vector.scalar_tensor_tensor(out=rrow, in0=tokio[:, tb:tb + 1],
                                                   scalar=float(NNB), in1=bstar,
                                                   op0=OP.mult, op1=OP.add)
                    ri = pBs.tile([128, 1], i32, tag="ri")
                    nc.vector.tensor_copy(out=ri, in_=rrow)
                    gl = pBs.tile([128, 512], f32, tag="gl")
                    gth = nc.gpsimd.indirect_dma_start(
                        out=gl[:, :], out_offset=None, in_=logits_rows,
                        in_offset=bass.IndirectOffsetOnAxis(ap=ri[:, :1], axis=0))
                    for dmi in logit_dmas[tb]:
                        add_dep_helper(gth.ins, dmi.ins, reason="gather logits after writeback")
                    psl = pBs.tile([128, 512], f32, tag="psl")
                    nc.vector.scalar_tensor_tensor(out=psl, in0=gl, scalar=mx, in1=iota512,
                                                   op0=OP.is_equal, op1=OP.mult)
                    pv_ = pBs.tile([128, 1], f32, tag="pv_")
                    nc.vector.tensor_reduce(out=pv_, in_=psl, axis=AX, op=OP.max)
                    pos = pBs.tile([128, 1], f32, tag="pos")
                    nc.vector.tensor_scalar(out=pos, in0=pv_, scalar1=511.0, scalar2=-1.0,
                                            op0=OP.subtract, op1=OP.mult)
                    idxf = pBs.tile([128, 1], f32, tag="idxf")
                    nc.vector.scalar_tensor_tensor(out=idxf, in0=bstar, scalar=512.0,
                                                   in1=pos, op0=OP.mult, op1=OP.add)
                    nc.sync.dma_start(out=idx_o[tb:tb + 1, :], in_=idxf[:, 0:1])
                    idxi = pBs.tile([128, 1], i32, tag="idxi")
                    nc.vector.tensor_copy(out=idxi, in_=idxf)
                    # quantized = codebook[idx]
                    qg = pBs.tile([128, D], f32, tag="qg")
                    nc.gpsimd.indirect_dma_start(
                        out=qg[:, :], out_offset=None, in_=cb[:, :],
                        in_offset=bass.IndirectOffsetOnAxis(ap=idxi[:, :1], axis=0))
                    nc.gpsimd.dma_start(out=quant_o[ts_, :], in_=qg)
                    # vq loss partials: sum_d (q - enh)^2 in transposed layout
                    for c in range(DC):
                        qt_ps = qps.tile([128, 128], f32, tag="qt")
                        nc.tensor.matmul(out=qt_ps[:, :], lhsT=qg[:, c * 128:(c + 1) * 128],
                                         rhs=ident[:, :], start=True, stop=True)
                        df = pBs.tile([128, 128], f32, tag="df")
                        nc.vector.tensor_tensor(out=df, in0=qt_ps, in1=EhT[:, c, ts_], op=OP.subtract)
                        nc.vector.tensor_tensor(out=df, in0=df, in1=ElT[:, c, ts_], op=OP.subtract)
                        sqs = pBs.tile([128, 128], f32, tag="sqs")
                        nc.scalar.activation(out=sqs, in_=df, func=AF.Square,
                                             bias=0.0, scale=1.0,
                                             accum_out=lossp[:, tb * DC + c:tb * DC + c + 1])

                sweep(0)
                # sweep 1 with sweep-0 finalizes interleaved at nb boundaries
                # slots 0 and 1 are rewritten by tb6/tb7 from nb=0, so both
                # of their previous owners must be finalized before any
                # sweep-1 tile is emitted
                fc_sched = {0: [0, 1], 4: [2], 8: [3], 12: [4]}
                sweep(1, fc_sched)
                for tb in SWEEP_TBS[1]:
                    finalize(tb)

                lsum = pBs.tile([128, 1], f32, tag="lsum")
                nc.vector.tensor_reduce(out=lsum, in_=lossp, axis=AX, op=OP.add)
                nc.sync.dma_start(out=loss_o[:, :], in_=lsum)
            pWL_cm.__exit__(None, None, None)

    nc.finalize()
    return nc


# ----------------------------- host side -----------------------------

_NC_CACHE = {}


def _get_nc(with_bias: bool):
    if with_bias not in _NC_CACHE:
        _NC_CACHE[with_bias] = build_nc(with_bias)
    return _NC_CACHE[with_bias]


def _split(a):
    a = np.ascontiguousarray(a, dtype=np.float32)
    h = a.astype(_BF)
    l = (a - h.astype(np.float32)).astype(_BF)
    return h, l


def _prep(inputs):
    x = np.asarray(inputs["x"], np.float32)
    with_bias = any(
        np.any(np.asarray(inputs[k])) for k in ("b_low", "b_mid", "b_high", "b_gate", "b_logits"))

    W = {}
    for e, nm in enumerate(("W_low", "W_mid", "W_high")):
        h, l = _split(np.asarray(inputs[nm]))
        W[f"W{e}h"] = h
        W[f"W{e}l"] = l
    W["Wgh"], W["Wgl"] = _split(np.asarray(inputs["W_gate"]))
    W["WLh"], W["WLl"] = _split(np.asarray(inputs["W_logits"]))
    vb = np.asarray(inputs["vehicle_emb"], np.float32).mean(axis=0)
    W["vbar"] = np.ascontiguousarray(vb.reshape(DC, 128).T, np.float32)
    W["cb"] = np.ascontiguousarray(np.asarray(inputs["codebook"], np.float32))
    if with_bias:
        bh, bl = _split(np.stack([np.asarray(inputs[k], np.float32)
                                  for k in ("b_low", "b_mid", "b_high")]))
        W["bexph"], W["bexpl"] = bh, bl
        gh, gl = _split(np.asarray(inputs["b_gate"], np.float32)[None, :])
        W["bgh"], W["bgl"] = gh, gl
        lh, ll = _split(np.asarray(inputs["b_logits"], np.float32)[None, :])
        W["blh"], W["bll"] = lh, ll

    in_maps = []
    for c in range(B):
        xT = np.ascontiguousarray(x[c].T)
        xh, xl = _split(xT)
        m = {"xTh": xh, "xTl": xl}
        m.update(W)
        in_maps.append(m)
    return in_maps, with_bias


def _run(inputs, trace=False):
    from concourse.bass_utils import run_bass_kernel_spmd
    in_maps, with_bias = _prep(inputs)
    nc = _get_nc(with_bias)
    res = run_bass_kernel_spmd(nc, in_maps, list(range(B)), trace=trace)
    return res


def _assemble(res):
    quant = np.stack([res.results[c]["quant_o"] for c in range(B)])
    logits = np.stack([res.results[c]["logits_o"] for c in range(B)])
    soft = np.stack([res.results[c]["soft_o"] for c in range(B)])
    idx = np.stack([np.rint(res.results[c]["idx_o"].reshape(T)).astype(np.int32)
                    for c in range(B)])
    tot = sum(float(res.results[c]["loss_o"].sum(dtype=np.float64)) for c in range(B))
    vq_loss = np.float32(tot / (B * T * D))
    return quant, idx, vq_loss, soft, logits


def kernel(**inputs):
    res = _run(inputs, trace=False)
    return _assemble(res)


def model_time_ns(with_bias=False):
    """Cost-model execution time (no NTFF profiling available under this
    axon container, so CoreSim's instruction cost model is the ns source)."""
    import concourse.bass_interp as bass_interp
    nc = _get_nc(with_bias)
    sim = bass_interp.CoreSim(nc, no_exec=True, publish_trace=False)
    sim.simulate()
    return int(sim.time)


def kernel_profiled(**inputs):
    res = _run(inputs, trace=False)
    out = _assemble(res)
    return out, model_time_ns()


# revision 19
# speedup vs baseline: 1.0775x; 1.0106x over previous
"""TRN2 Bass kernel for the BEATs-style VQ tokenizer (vq_codebook problem).

Data-parallel over batch B=8 across 8 NeuronCores. Each core processes its
1024-token slice end to end:

  phase A: gate (token layout, bf16x3 matmuls + DVE polynomial exp softmax),
           three expert matmuls in transposed [d, token] layout (bf16x3),
           gated combine via PE broadcast-transposes + DVE, vehicle-mean add,
           then split enhancedT into bf16 hi/lo for the logits matmuls.
  phase B: logits = enhancedT.T @ W_logits as bf16x3 (hi*hi + hi*lo + lo*hi),
           tiled [128 tok, 512 cb] psum tiles; per tile: ScalarE copy to SBUF
           (-> HBM logits out), ScalarE exp (fp16, accum row-sums), VectorE
           block-max. Two sweeps of W_logits (4 token blocks each) keep the
           fp16 exp buffer within SBUF.
  finalize per token block: softmax scale (Newton-refined reciprocal),
           exact argmax (block max -> winning block gather from HBM ->
           position via iota/is_equal), codebook gather, vq-loss partials.

Precision: all matmuls are bf16 hi/lo split x3 (exact products, f32 psum
accumulation) giving ~4e-5 absmax logit error vs the f32 reference, far under
the 1.8e-5..~0.1 top-2 logit gaps -> argmax matches the reference exactly.
quantized == codebook[idx] holds bitwise in the reference (verified).
"""

import numpy as np
import ml_dtypes

import concourse.bass as bass
import concourse.bacc as bacc
import concourse.mybir as mybir
import concourse.tile as tile
from concourse.tile_rust import add_dep_helper

B, T, D, KCB = 8, 1024, 768, 8192
NTB = 8          # token blocks of 128 per core
NNB = 16         # codebook-dim blocks of 512
DC = 6           # d chunks of 128
SWEEP_TBS = [[0, 1, 2], [3, 4, 5], [6, 7]]  # W_logits sweeps (3/3/2 split)

f32 = mybir.dt.float32
f16 = mybir.dt.float16
bf16 = mybir.dt.bfloat16
i32 = mybir.dt.int32
AX = mybir.AxisListType.X
OP = mybir.AluOpType
AF = mybir.ActivationFunctionType

_BF = ml_dtypes.bfloat16


def _exp_poly_coeffs():
    """Power-basis coeffs (in u = x/4 + 2, u in [-2,2]) approximating
    e^(u-2); gate exp is then poly(u)^4 = e^x for x in [-16, 0]."""
    u = np.linspace(-2, 2, 20001)
    cheb = np.polynomial.chebyshev.Chebyshev.fit(u, np.exp(u - 2.0), deg=15)
    c = cheb.convert(kind=np.polynomial.Polynomial).coef
    rel = np.abs(np.polyval(c[::-1], u) / np.exp(u - 2.0) - 1.0).max()
    assert rel < 1e-7, rel
    return [float(v) for v in c]  # c[k] multiplies u^k

_EXP_C = _exp_poly_coeffs()


def build_nc(with_bias: bool):
    nc = bacc.Bacc(None)

    # ---- inputs (per core) ----
    xTh = nc.dram_tensor("xTh", [D, T], bf16, kind="ExternalInput")
    xTl = nc.dram_tensor("xTl", [D, T], bf16, kind="ExternalInput")
    Wh = [nc.dram_tensor(f"W{e}h", [D, D], bf16, kind="ExternalInput") for e in range(3)]
    Wl = [nc.dram_tensor(f"W{e}l", [D, D], bf16, kind="ExternalInput") for e in range(3)]
    Wgh = nc.dram_tensor("Wgh", [D, 3], bf16, kind="ExternalInput")
    Wgl = nc.dram_tensor("Wgl", [D, 3], bf16, kind="ExternalInput")
    WLh = nc.dram_tensor("WLh", [D, KCB], bf16, kind="ExternalInput")
    WLl = nc.dram_tensor("WLl", [D, KCB], bf16, kind="ExternalInput")
    vbar = nc.dram_tensor("vbar", [128, DC], f32, kind="ExternalInput")
    cb = nc.dram_tensor("cb", [KCB, D], f32, kind="ExternalInput")
    if with_bias:
        bexph = nc.dram_tensor("bexph", [3, D], bf16, kind="ExternalInput")
        bexpl = nc.dram_tensor("bexpl", [3, D], bf16, kind="ExternalInput")
        bgh = nc.dram_tensor("bgh", [1, 3], bf16, kind="ExternalInput")
        bgl = nc.dram_tensor("bgl", [1, 3], bf16, kind="ExternalInput")
        blh = nc.dram_tensor("blh", [1, KCB], bf16, kind="ExternalInput")
        bll = nc.dram_tensor("bll", [1, KCB], bf16, kind="ExternalInput")

    # ---- outputs (per core) ----
    logits_o = nc.dram_tensor("logits_o", [T, KCB], f32, kind="ExternalOutput")
    soft_o = nc.dram_tensor("soft_o", [T, KCB], f32, kind="ExternalOutput")
    quant_o = nc.dram_tensor("quant_o", [T, D], f32, kind="ExternalOutput")
    idx_o = nc.dram_tensor("idx_o", [NTB, 128], f32, kind="ExternalOutput")
    loss_o = nc.dram_tensor("loss_o", [128, 1], f32, kind="ExternalOutput")

    logits_rows = logits_o[:, :].rearrange("t (b c) -> (t b) c", c=512)

    with tile.TileContext(nc) as tc:
        with tc.tile_pool(name="pers", bufs=1) as pers:
            # persistent tiles
            EhT = pers.tile([128, DC, T], bf16)
            ElT = pers.tile([128, DC, T], bf16)
            spart = pers.tile([128, NTB, NNB], f32)
            mpart = pers.tile([128, NTB, NNB], f32)
            lossp = pers.tile([128, NTB * DC], f32)
            iota512 = pers.tile([128, 512], f32)
            iota16 = pers.tile([128, NNB], f32)
            ident = pers.tile([128, 128], f32)
            vbar_t = pers.tile([128, DC], f32)
            tokio = pers.tile([128, NTB], f32)
            ones_h = pers.tile([1, T], bf16)

            # constants
            it512 = pers.tile([128, 512], i32)
            nc.gpsimd.iota(it512, pattern=[[-1, 512]], base=511, channel_multiplier=0)
            nc.vector.tensor_copy(out=iota512, in_=it512)
            it16 = pers.tile([128, NNB], i32)
            nc.gpsimd.iota(it16, pattern=[[-1, NNB]], base=NNB - 1, channel_multiplier=0)
            nc.vector.tensor_copy(out=iota16, in_=it16)
            itok = pers.tile([128, NTB], i32)
            nc.gpsimd.iota(itok, pattern=[[128, NTB]], base=0, channel_multiplier=1)
            nc.vector.tensor_copy(out=tokio, in_=itok)
            from concourse.masks import make_identity
            make_identity(nc, ident)
            nc.sync.dma_start(out=vbar_t, in_=vbar[:, :])
            nc.vector.memset(ones_h, 1.0)

            # =========================== phase A ===========================
            pWL_cm = tc.tile_pool(name="pWL", bufs=3)
            pWL = pWL_cm.__enter__()
            with tc.tile_pool(name="pA", bufs=1) as pA:
                xh_t = pA.tile([128, DC, T], bf16)
                xl_t = pA.tile([128, DC, T], bf16)
                for c in range(DC):
                    nc.sync.dma_start(out=xh_t[:, c, :], in_=xTh[c * 128:(c + 1) * 128, :])
                    nc.scalar.dma_start(out=xl_t[:, c, :], in_=xTl[c * 128:(c + 1) * 128, :])
                wgh_t = pA.tile([128, DC, 3], bf16)
                nc.sync.dma_start(out=wgh_t, in_=Wgh[:, :].rearrange("(c p) n -> p c n", p=128))
                wgl_t = pA.tile([128, DC, 3], bf16)
                nc.sync.dma_start(out=wgl_t, in_=Wgl[:, :].rearrange("(c p) n -> p c n", p=128))
                if with_bias:
                    bgh_t = pA.tile([1, 3], bf16)
                    nc.sync.dma_start(out=bgh_t, in_=bgh[:, :])
                    bgl_t = pA.tile([1, 3], bf16)
                    nc.sync.dma_start(out=bgl_t, in_=bgl[:, :])
                    xon = pA.tile([1, T], bf16)
                    nc.vector.memset(xon, 1.0)
                    bexph_t = pA.tile([3, D], bf16)
                    nc.sync.dma_start(out=bexph_t, in_=bexph[:, :])
                    bexpl_t = pA.tile([3, D], bf16)
                    nc.sync.dma_start(out=bexpl_t, in_=bexpl[:, :])

                # ---- gate logits, token layout [128 tok, 3] per tb ----
                glog = pA.tile([128, NTB, 3], f32)
                with tc.tile_pool(name="gps", bufs=2, space="PSUM") as gps:
                    for tb in range(NTB):
                        g_ps = gps.tile([128, 3], f32, tag="g")
                        first = True
                        for si, (xs, ws) in enumerate(((xh_t, wgh_t), (xh_t, wgl_t), (xl_t, wgh_t))):
                            for c in range(DC):
                                last = (not with_bias) and si == 2 and c == DC - 1
                                nc.tensor.matmul(
                                    out=g_ps[:, :],
                                    lhsT=xs[:, c, tb * 128:(tb + 1) * 128],
                                    rhs=ws[:, c, :],
                                    start=first, stop=last)
                                first = False
                        if with_bias:
                            nc.tensor.matmul(out=g_ps[:, :], lhsT=xon[0:1, tb * 128:(tb + 1) * 128],
                                             rhs=bgh_t[0:1, :], start=False, stop=False)
                            nc.tensor.matmul(out=g_ps[:, :], lhsT=xon[0:1, tb * 128:(tb + 1) * 128],
                                             rhs=bgl_t[0:1, :], start=False, stop=True)
                        nc.scalar.copy(out=glog[:, tb, :], in_=g_ps[:, :])

                # ---- gate softmax: shift by max, DVE polynomial exp ----
                cent = pA.tile([128, NTB, 3], f32)
                for tb in range(NTB):
                    nm = pA.tile([128, 1], f32, tag="gnm")
                    nc.vector.tensor_reduce(out=nm, in_=glog[:, tb, :], axis=AX,
                                            op=OP.max, negate=True)
                    nc.vector.tensor_scalar(out=cent[:, tb, :], in0=glog[:, tb, :],
                                            scalar1=nm, scalar2=None, op0=OP.add)
                flat = cent[:, :, :].rearrange("p a b -> p (a b)")
                u = pA.tile([128, NTB * 3], f32)
                # u = max(x, -16) * 0.25 + 2
                nc.vector.tensor_scalar(out=u, in0=flat, scalar1=-16.0, scalar2=0.25,
                                        op0=OP.max, op1=OP.mult)
                nc.vector.tensor_scalar(out=u, in0=u, scalar1=2.0, scalar2=None, op0=OP.add)
                pv = pA.tile([128, NTB * 3], f32)
                nc.vector.memset(pv, 0.0)
                for k in range(15, 0, -1):
                    nc.vector.scalar_tensor_tensor(out=pv, in0=pv, scalar=float(_EXP_C[k]),
                                                   in1=u, op0=OP.add, op1=OP.mult)
                nc.vector.tensor_scalar(out=pv, in0=pv, scalar1=float(_EXP_C[0]),
                                        scalar2=None, op0=OP.add)
                nc.vector.tensor_tensor(out=pv, in0=pv, in1=pv, op=OP.mult)
                nc.vector.tensor_tensor(out=pv, in0=pv, in1=pv, op=OP.mult)
                gexp = pv.rearrange("p (a b) -> p a b", b=3)

                # row sums + newton reciprocal -> rs [128, NTB]
                gs = pA.tile([128, NTB], f32)
                nc.vector.tensor_reduce(out=gs, in_=gexp, axis=AX, op=OP.add)
                rs0 = pA.tile([128, NTB], f32)
                nc.vector.reciprocal(out=rs0, in_=gs)
                tnw = pA.tile([128, NTB], f32)
                nc.vector.tensor_tensor(out=tnw, in0=gs, in1=rs0, op=OP.mult)
                nc.vector.tensor_scalar(out=tnw, in0=tnw, scalar1=2.0, scalar2=-1.0,
                                        op0=OP.subtract, op1=OP.mult)
                rs = pA.tile([128, NTB], f32)
                nc.vector.tensor_tensor(out=rs, in0=rs0, in1=tnw, op=OP.mult)

                # ---- broadcast g0/g1/g2 and 1/s across partitions ----
                gb_sb = [pA.tile([128, T], f32, tag=f"gb{e}", name=f"gb{e}") for e in range(3)]
                rb_sb = pA.tile([128, T], f32, tag="rb")
                with tc.tile_pool(name="bps", bufs=3, space="PSUM") as bps:
                    for tb in range(NTB):
                        for e in range(3):
                            t_ps = bps.tile([128, 128], f32, tag="bc")
                            nc.tensor.transpose(
                                out=t_ps[:, :],
                                in_=gexp[:, tb, e:e + 1].to_broadcast([128, 128]),
                                identity=ident[:, :])
                            nc.scalar.copy(out=gb_sb[e][:, tb * 128:(tb + 1) * 128], in_=t_ps)
                        t_ps = bps.tile([128, 128], f32, tag="bc")
                        nc.tensor.transpose(
                            out=t_ps[:, :],
                            in_=rs[:, tb:tb + 1].to_broadcast([128, 128]),
                            identity=ident[:, :])
                        nc.scalar.copy(out=rb_sb[:, tb * 128:(tb + 1) * 128], in_=t_ps)

                # ---- experts (transposed layout) + gated combine ----
                enhT = pA.tile([128, DC, T], f32)
                with tc.tile_pool(name="pw", bufs=2) as pw, \
                     tc.tile_pool(name="eps", bufs=2, space="PSUM") as eps:
                    for co in range(DC):
                        wt = {}
                        for e in range(3):
                            wt[(e, "h")] = pw.tile([128, DC, 128], bf16, tag=f"w{e}h", name=f"w{e}h")
                            nc.sync.dma_start(
                                out=wt[(e, "h")],
                                in_=Wh[e][:, co * 128:(co + 1) * 128].rearrange("(c p) n -> p c n", p=128))
                            wt[(e, "l")] = pw.tile([128, DC, 128], bf16, tag=f"w{e}l", name=f"w{e}l")
                            nc.scalar.dma_start(
                                out=wt[(e, "l")],
                                in_=Wl[e][:, co * 128:(co + 1) * 128].rearrange("(c p) n -> p c n", p=128))
                        for half in range(2):
                            hs = slice(half * 512, (half + 1) * 512)
                            e_ps = []
                            for e in range(3):
                                ps_ = eps.tile([128, 512], f32, tag=f"e{e}")
                                first = True
                                for (xs, wk) in ((xh_t, "h"), (xh_t, "l"), (xl_t, "h")):
                                    for c in range(DC):
                                        last = (not with_bias) and xs is xl_t and c == DC - 1
                                        nc.tensor.matmul(
                                            out=ps_[:, :], lhsT=wt[(e, wk)][:, c, :],
                                            rhs=xs[:, c, hs], start=first, stop=last)
                                        first = False
                                if with_bias:
                                    nc.tensor.matmul(
                                        out=ps_[:, :], lhsT=bexph_t[e:e + 1, co * 128:(co + 1) * 128],
                                        rhs=xon[0:1, hs], start=False, stop=False)
                                    nc.tensor.matmul(
                                        out=ps_[:, :], lhsT=bexpl_t[e:e + 1, co * 128:(co + 1) * 128],
                                        rhs=xon[0:1, hs], start=False, stop=True)
                                e_ps.append(ps_)
                            uacc = pA.tile([128, 512], f32, tag="uacc")
                            vtmp = pA.tile([128, 512], f32, tag="vtmp")
                            nc.vector.tensor_tensor(out=uacc, in0=e_ps[0], in1=gb_sb[0][:, hs], op=OP.mult)
                            nc.vector.tensor_tensor(out=vtmp, in0=e_ps[1], in1=gb_sb[1][:, hs], op=OP.mult)
                            nc.vector.tensor_tensor(out=uacc, in0=uacc, in1=vtmp, op=OP.add)
                            nc.vector.tensor_tensor(out=vtmp, in0=e_ps[2], in1=gb_sb[2][:, hs], op=OP.mult)
                            nc.vector.tensor_tensor(out=uacc, in0=uacc, in1=vtmp, op=OP.add)
                            nc.vector.tensor_tensor(out=uacc, in0=uacc, in1=rb_sb[:, hs], op=OP.mult)
                            nc.vector.tensor_scalar(out=enhT[:, co, hs], in0=uacc,
                                                    scalar1=vbar_t[:, co:co + 1], scalar2=None,
                                                    op0=OP.add)
                        # split this chunk of enhancedT into bf16 hi/lo right
                        # away so phase B can start as soon as all chunks land
                        nc.vector.tensor_copy(out=EhT[:, co, :], in_=enhT[:, co, :])
                        nc.vector.tensor_tensor(out=ElT[:, co, :], in0=enhT[:, co, :],
                                                in1=EhT[:, co, :], op=OP.subtract)

            # =========================== phase B ===========================
            with tc.tile_pool(name="pB", bufs=1) as pB, \
                 tc.tile_pool(name="pBs", bufs=3) as pBs, \
                 tc.tile_pool(name="lps", bufs=6, space="PSUM") as lps, \
                 tc.tile_pool(name="qps", bufs=2, space="PSUM") as qps:
                NSLOT = 4  # ring of exp slots (tb % 4)
                expbuf = pB.tile([128, NSLOT, KCB], f16)
                logit_dmas = {tb: [] for tb in range(NTB)}

                def sweep(sw, fc_sched=None):
                    tbs = SWEEP_TBS[sw]
                    for nb in range(NNB):
                        if fc_sched and nb in fc_sched:
                            for _tb in fc_sched[nb]:
                                finalize(_tb)
                        ns = slice(nb * 512, (nb + 1) * 512)
                        wlh_t = pWL.tile([128, DC, 512], bf16, tag="wlh")
                        nc.sync.dma_start(out=wlh_t, in_=WLh[:, ns].rearrange("(c p) n -> p c n", p=128))
                        wll_t = pWL.tile([128, DC, 512], bf16, tag="wll")
                        nc.scalar.dma_start(out=wll_t, in_=WLl[:, ns].rearrange("(c p) n -> p c n", p=128))
                        if with_bias:
                            blh_t = pBs.tile([1, 512], bf16, tag="blh")
                            nc.sync.dma_start(out=blh_t, in_=blh[:, ns])
                            bll_t = pBs.tile([1, 512], bf16, tag="bll")
                            nc.sync.dma_start(out=bll_t, in_=bll[:, ns])
                        for tb in tbs:
                            ts_ = slice(tb * 128, (tb + 1) * 128)
                            lp = lps.tile([128, 512], f32, tag="lp")
                            first = True
                            for (es, ws) in ((EhT, wlh_t), (EhT, wll_t), (ElT, wlh_t)):
                                for c in range(DC):
                                    last = (not with_bias) and es is ElT and c == DC - 1
                                    nc.tensor.matmul(out=lp[:, :], lhsT=es[:, c, ts_],
                                                     rhs=ws[:, c, :], start=first, stop=last)
                                    first = False
                            if with_bias:
                                nc.tensor.matmul(out=lp[:, :], lhsT=ones_h[0:1, ts_],
                                                 rhs=blh_t[0:1, :], start=False, stop=False)
                                nc.tensor.matmul(out=lp[:, :], lhsT=ones_h[0:1, ts_],
                                                 rhs=bll_t[0:1, :], start=False, stop=True)
                            stg = pBs.tile([128, 512], f32, tag="lstg")
                            nc.scalar.copy(out=stg, in_=lp)
                            dmi = nc.scalar.dma_start(out=logits_o[ts_, ns], in_=stg)
                            logit_dmas[tb].append(dmi)
                            nc.scalar.activation(
                                out=expbuf[:, tb % NSLOT, ns], in_=lp, func=AF.Exp,
                                bias=0.0, scale=1.0,
                                accum_out=spart[:, tb, nb:nb + 1])
                            nc.vector.tensor_reduce(out=mpart[:, tb, nb:nb + 1], in_=lp,
                                                    axis=AX, op=OP.max)

                def finalize(tb):
                    ts_ = slice(tb * 128, (tb + 1) * 128)
                    lt = tb % NSLOT
                    # softmax scale r = 1/S (newton)
                    s1 = pBs.tile([128, 1], f32, tag="s1")
                    nc.vector.tensor_reduce(out=s1, in_=spart[:, tb, :], axis=AX, op=OP.add)
                    r0 = pBs.tile([128, 1], f32, tag="r0")
                    nc.vector.reciprocal(out=r0, in_=s1)
                    tn = pBs.tile([128, 1], f32, tag="tn")
                    nc.vector.tensor_tensor(out=tn, in0=s1, in1=r0, op=OP.mult)
                    nc.vector.tensor_scalar(out=tn, in0=tn, scalar1=2.0, scalar2=-1.0,
                                            op0=OP.subtract, op1=OP.mult)
                    rr = pBs.tile([128, 1], f32, tag="rr")
                    nc.vector.tensor_tensor(out=rr, in0=r0, in1=tn, op=OP.mult)
                    # soft = expbuf * r first: frees this tb's exp slot ASAP
                    for ch in range(4):
                        cs = slice(ch * 2048, (ch + 1) * 2048)
                        sst = pBs.tile([128, 2048], f32, tag="sst", name="sst", bufs=2)
                        nc.vector.tensor_scalar(out=sst, in0=expbuf[:, lt, cs],
                                                scalar1=rr, scalar2=None, op0=OP.mult)
                        nc.sync.dma_start(out=soft_o[ts_, cs], in_=sst)
                    # argmax: row max + winning block
                    mx = pBs.tile([128, 1], f32, tag="mx")
                    nc.vector.tensor_reduce(out=mx, in_=mpart[:, tb, :], axis=AX, op=OP.max)
                    bsl = pBs.tile([128, NNB], f32, tag="bsl")
                    nc.vector.scalar_tensor_tensor(out=bsl, in0=mpart[:, tb, :], scalar=mx,
                                                   in1=iota16, op0=OP.is_equal, op1=OP.mult)
                    bv = pBs.tile([128, 1], f32, tag="bv")
                    nc.vector.tensor_reduce(out=bv, in_=bsl, axis=AX, op=OP.max)
                    bstar = pBs.tile([128, 1], f32, tag="bstar")
                    nc.vector.tensor_scalar(out=bstar, in0=bv, scalar1=float(NNB - 1),
                                            scalar2=-1.0, op0=OP.subtract, op1=OP.mult)
                    # gather the winning 512-block of this tb's logits rows
                    rrow = pBs.tile([128, 1], f32, tag="rrow")
                    nc.vector.scalar_tensor_tensor(out=rrow, in0=tokio[:, tb:tb + 1],
                                                   scalar=float(NNB), in1=bstar,
                                                   op0=OP.mult, op1=OP.add)
                    ri = pBs.tile([128, 1], i32, tag="ri")
                    nc.vector.tensor_copy(out=ri, in_=rrow)
                    gl = pBs.tile([128, 512], f32, tag="gl")
                    gth = nc.gpsimd.indirect_dma_start(
                        out=gl[:, :], out_offset=None, in_=logits_rows,
                        in_offset=bass.IndirectOffsetOnAxis(ap=ri[:, :1], axis=0))
                    for dmi in logit_dmas[tb]:
                        add_dep_helper(gth.ins, dmi.ins, reason="gather logits after writeback")
                    psl = pBs.tile([128, 512], f32, tag="psl")
                    nc.vector.scalar_tensor_tensor(out=psl, in0=gl, scalar=mx, in1=iota512,
                                                   op0=OP.is_equal, op1=OP.mult)
                    pv_ = pBs.tile([128, 1], f32, tag="pv_")
                    nc.vector.tensor_reduce(out=pv_, in_=psl, axis=AX, op=OP.max)
                    pos = pBs.tile([128, 1], f32, tag="pos")
                    nc.vector.tensor_scalar(out=pos, in0=pv_, scalar1=511.0, scalar2=-1.0,
                                            op0=OP.subtract, op1=OP.mult)
                    idxf = pBs.tile([128, 1], f32, tag="idxf")
                    nc.vector.scalar_tensor_tensor(out=idxf, in0=bstar, scalar=512.0,
                                                   in1=pos, op0=OP.mult, op1=OP.add)
                    nc.sync.dma_start(out=idx_o[tb:tb + 1, :], in_=idxf[:, 0:1])
                    idxi = pBs.tile([128, 1], i32, tag="idxi")
                    nc.vector.tensor_copy(out=idxi, in_=idxf)
                    # quantized = codebook[idx]
                    qg = pBs.tile([128, D], f32, tag="qg")
                    nc.gpsimd.indirect_dma_start(
                        out=qg[:, :], out_offset=None, in_=cb[:, :],
                        in_offset=bass.IndirectOffsetOnAxis(ap=idxi[:, :1], axis=0))
                    nc.gpsimd.dma_start(out=quant_o[ts_, :], in_=qg)
                    # vq loss partials: sum_d (q - enh)^2 in transposed layout
                    for c in range(DC):
                        qt_ps = qps.tile([128, 128], f32, tag="qt")
                        nc.tensor.matmul(out=qt_ps[:, :], lhsT=qg[:, c * 128:(c + 1) * 128],
                                         rhs=ident[:, :], start=True, stop=True)
                        df = pBs.tile([128, 128], f32, tag="df")
                        nc.vector.tensor_tensor(out=df, in0=qt_ps, in1=EhT[:, c, ts_], op=OP.subtract)
                        nc.vector.tensor_tensor(out=df, in0=df, in1=ElT[:, c, ts_], op=OP.subtract)
                        sqs = pBs.tile([128, 128], f32, tag="sqs")
                        nc.scalar.activation(out=sqs, in_=df, func=AF.Square,
                                             bias=0.0, scale=1.0,
                                             accum_out=lossp[:, tb * DC + c:tb * DC + c + 1])

                sweep(0)
                # interleave earlier sweeps' finalizes into later sweeps.
                # a finalize must be emitted before the first tile write of
                # any sweep that reuses its exp ring slot (slot = tb % 4):
                # sweep1 (tb 3,4,5 -> slots 3,0,1) reuses tb0/tb1's slots;
                # sweep2 (tb 6,7 -> slots 2,3) reuses tb2/tb3's slots.
                sweep(1, {0: [0, 1], 8: [2]})
                sweep(2, {0: [3], 5: [4], 10: [5]})
                for tb in SWEEP_TBS[2]:
                    finalize(tb)

                lsum = pBs.tile([128, 1], f32, tag="lsum")
                nc.vector.tensor_reduce(out=lsum, in_=lossp, axis=AX, op=OP.add)
                nc.sync.dma_start(out=loss_o[:, :], in_=lsum)
            pWL_cm.__exit__(None, None, None)

    nc.finalize()
    return nc


# ----------------------------- host side -----------------------------

_NC_CACHE = {}


def _get_nc(with_bias: bool):
    if with_bias not in _NC_CACHE:
        _NC_CACHE[with_bias] = build_nc(with_bias)
    return _NC_CACHE[with_bias]


def _split(a):
    a = np.ascontiguousarray(a, dtype=np.float32)
    h = a.astype(_BF)
    l = (a - h.astype(np.float32)).astype(_BF)
    return h, l


def _prep(inputs):
    x = np.asarray(inputs["x"], np.float32)
    with_bias = any(
        np.any(np.asarray(inputs[k])) for k in ("b_low", "b_mid", "b_high", "b_gate", "b_logits"))

    W = {}
    for e, nm in enumerate(("W_low", "W_mid", "W_high")):
        h, l = _split(np.asarray(inputs[nm]))
        W[f"W{e}h"] = h
        W[f"W{e}l"] = l
    W["Wgh"], W["Wgl"] = _split(np.asarray(inputs["W_gate"]))
    W["WLh"], W["WLl"] = _split(np.asarray(inputs["W_logits"]))
    vb = np.asarray(inputs["vehicle_emb"], np.float32).mean(axis=0)
    W["vbar"] = np.ascontiguousarray(vb.reshape(DC, 128).T, np.float32)
    W["cb"] = np.ascontiguousarray(np.asarray(inputs["codebook"], np.float32))
    if with_bias:
        bh, bl = _split(np.stack([np.asarray(inputs[k], np.float32)
                                  for k in ("b_low", "b_mid", "b_high")]))
        W["bexph"], W["bexpl"] = bh, bl
        gh, gl = _split(np.asarray(inputs["b_gate"], np.float32)[None, :])
        W["bgh"], W["bgl"] = gh, gl
        lh, ll = _split(np.asarray(inputs["b_logits"], np.float32)[None, :])
        W["blh"], W["bll"] = lh, ll

    in_maps = []
    for c in range(B):
        xT = np.ascontiguousarray(x[c].T)
        xh, xl = _split(xT)
        m = {"xTh": xh, "xTl": xl}
        m.update(W)
        in_maps.append(m)
    return in_maps, with_bias


def _run(inputs, trace=False):
    from concourse.bass_utils import run_bass_kernel_spmd
    in_maps, with_bias = _prep(inputs)
    nc = _get_nc(with_bias)
    res = run_bass_kernel_spmd(nc, in_maps, list(range(B)), trace=trace)
    return res


def _assemble(res):
    quant = np.stack([res.results[c]["quant_o"] for c in range(B)])
    logits = np.stack([res.results[c]["logits_o"] for c in range(B)])
    soft = np.stack([res.results[c]["soft_o"] for c in range(B)])
    idx = np.stack([np.rint(res.results[c]["idx_o"].reshape(T)).astype(np.int32)
                    for c in range(B)])
    tot = sum(float(res.results[c]["loss_o"].sum(dtype=np.float64)) for c in range(B))
    vq_loss = np.float32(tot / (B * T * D))
    return quant, idx, vq_loss, soft, logits


def kernel(**inputs):
    res = _run(inputs, trace=False)
    return _assemble(res)


def model_time_ns(with_bias=False):
    """Cost-model execution time (no NTFF profiling available under this
    axon container, so CoreSim's instruction cost model is the ns source)."""
    import concourse.bass_interp as bass_interp
    nc = _get_nc(with_bias)
    sim = bass_interp.CoreSim(nc, no_exec=True, publish_trace=False)
    sim.simulate()
    return int(sim.time)


def kernel_profiled(**inputs):
    res = _run(inputs, trace=False)
    out = _assemble(res)
    return out, model_time_ns()


# revision 22
# speedup vs baseline: 1.0812x; 1.0034x over previous
"""TRN2 Bass kernel for the BEATs-style VQ tokenizer (vq_codebook problem).

Data-parallel over batch B=8 across 8 NeuronCores. Each core processes its
1024-token slice end to end:

  phase A: gate (token layout, bf16x3 matmuls + DVE polynomial exp softmax),
           three expert matmuls in transposed [d, token] layout (bf16x3),
           gated combine via PE broadcast-transposes + DVE, vehicle-mean add,
           then split enhancedT into bf16 hi/lo for the logits matmuls.
  phase B: logits = enhancedT.T @ W_logits as bf16x3 (hi*hi + hi*lo + lo*hi),
           tiled [128 tok, 512 cb] psum tiles; per tile: ScalarE copy to SBUF
           (-> HBM logits out), ScalarE exp (fp16, accum row-sums), VectorE
           block-max. Two sweeps of W_logits (4 token blocks each) keep the
           fp16 exp buffer within SBUF.
  finalize per token block: softmax scale (Newton-refined reciprocal),
           exact argmax (block max -> winning block gather from HBM ->
           position via iota/is_equal), codebook gather, vq-loss partials.

Precision: all matmuls are bf16 hi/lo split x3 (exact products, f32 psum
accumulation) giving ~4e-5 absmax logit error vs the f32 reference, far under
the 1.8e-5..~0.1 top-2 logit gaps -> argmax matches the reference exactly.
quantized == codebook[idx] holds bitwise in the reference (verified).
"""

import numpy as np
import ml_dtypes

import concourse.bass as bass
import concourse.bacc as bacc
import concourse.mybir as mybir
import concourse.tile as tile
from concourse.tile_rust import add_dep_helper

B, T, D, KCB = 8, 1024, 768, 8192
NTB = 8          # token blocks of 128 per core
NNB = 16         # codebook-dim blocks of 512
DC = 6           # d chunks of 128
SWEEP_TBS = [[0, 1, 2], [3, 4, 5], [6, 7]]  # W_logits sweeps (3/3/2 split)

f32 = mybir.dt.float32
f16 = mybir.dt.float16
bf16 = mybir.dt.bfloat16
i32 = mybir.dt.int32
AX = mybir.AxisListType.X
OP = mybir.AluOpType
AF = mybir.ActivationFunctionType

_BF = ml_dtypes.bfloat16


def _exp_poly_coeffs():
    """Power-basis coeffs (in u = x/4 + 2, u in [-2,2]) approximating
    e^(u-2); gate exp is then poly(u)^4 = e^x for x in [-16, 0]."""
    u = np.linspace(-2, 2, 20001)
    cheb = np.polynomial.chebyshev.Chebyshev.fit(u, np.exp(u - 2.0), deg=15)
    c = cheb.convert(kind=np.polynomial.Polynomial).coef
    rel = np.abs(np.polyval(c[::-1], u) / np.exp(u - 2.0) - 1.0).max()
    assert rel < 1e-7, rel
    return [float(v) for v in c]  # c[k] multiplies u^k

_EXP_C = _exp_poly_coeffs()


def build_nc(with_bias: bool):
    nc = bacc.Bacc(None)

    # ---- inputs (per core) ----
    xTh = nc.dram_tensor("xTh", [D, T], bf16, kind="ExternalInput")
    xTl = nc.dram_tensor("xTl", [D, T], bf16, kind="ExternalInput")
    Wh = [nc.dram_tensor(f"W{e}h", [D, D], bf16, kind="ExternalInput") for e in range(3)]
    Wl = [nc.dram_tensor(f"W{e}l", [D, D], bf16, kind="ExternalInput") for e in range(3)]
    Wgh = nc.dram_tensor("Wgh", [D, 3], bf16, kind="ExternalInput")
    Wgl = nc.dram_tensor("Wgl", [D, 3], bf16, kind="ExternalInput")
    WLh = nc.dram_tensor("WLh", [D, KCB], bf16, kind="ExternalInput")
    WLl = nc.dram_tensor("WLl", [D, KCB], bf16, kind="ExternalInput")
    vbar = nc.dram_tensor("vbar", [128, DC], f32, kind="ExternalInput")
    cb = nc.dram_tensor("cb", [KCB, D], f32, kind="ExternalInput")
    if with_bias:
        bexph = nc.dram_tensor("bexph", [3, D], bf16, kind="ExternalInput")
        bexpl = nc.dram_tensor("bexpl", [3, D], bf16, kind="ExternalInput")
        bgh = nc.dram_tensor("bgh", [1, 3], bf16, kind="ExternalInput")
        bgl = nc.dram_tensor("bgl", [1, 3], bf16, kind="ExternalInput")
        blh = nc.dram_tensor("blh", [1, KCB], bf16, kind="ExternalInput")
        bll = nc.dram_tensor("bll", [1, KCB], bf16, kind="ExternalInput")

    # ---- outputs (per core) ----
    logits_o = nc.dram_tensor("logits_o", [T, KCB], f32, kind="ExternalOutput")
    soft_o = nc.dram_tensor("soft_o", [T, KCB], f32, kind="ExternalOutput")
    quant_o = nc.dram_tensor("quant_o", [T, D], f32, kind="ExternalOutput")
    idx_o = nc.dram_tensor("idx_o", [NTB, 128], f32, kind="ExternalOutput")
    loss_o = nc.dram_tensor("loss_o", [128, 1], f32, kind="ExternalOutput")

    logits_rows = logits_o[:, :].rearrange("t (b c) -> (t b) c", c=512)

    with tile.TileContext(nc) as tc:
        with tc.tile_pool(name="pers", bufs=1) as pers:
            # persistent tiles
            EhT = pers.tile([128, DC, T], bf16)
            ElT = pers.tile([128, DC, T], bf16)
            spart = pers.tile([128, NTB, NNB], f32)
            mpart = pers.tile([128, NTB, NNB], f32)
            lossp = pers.tile([128, NTB * DC], f32)
            iota512 = pers.tile([128, 512], f32)
            iota16 = pers.tile([128, NNB], f32)
            ident = pers.tile([128, 128], f32)
            vbar_t = pers.tile([128, DC], f32)
            tokio = pers.tile([128, NTB], f32)
            ones_h = pers.tile([1, T], bf16)

            # constants
            it512 = pers.tile([128, 512], i32)
            nc.gpsimd.iota(it512, pattern=[[-1, 512]], base=511, channel_multiplier=0)
            nc.vector.tensor_copy(out=iota512, in_=it512)
            it16 = pers.tile([128, NNB], i32)
            nc.gpsimd.iota(it16, pattern=[[-1, NNB]], base=NNB - 1, channel_multiplier=0)
            nc.vector.tensor_copy(out=iota16, in_=it16)
            itok = pers.tile([128, NTB], i32)
            nc.gpsimd.iota(itok, pattern=[[128, NTB]], base=0, channel_multiplier=1)
            nc.vector.tensor_copy(out=tokio, in_=itok)
            from concourse.masks import make_identity
            make_identity(nc, ident)
            nc.sync.dma_start(out=vbar_t, in_=vbar[:, :])
            nc.vector.memset(ones_h, 1.0)

            # =========================== phase A ===========================
            pWL_cm = tc.tile_pool(name="pWL", bufs=3)
            pWL = pWL_cm.__enter__()
            with tc.tile_pool(name="pA", bufs=1) as pA:
                xh_t = pA.tile([128, DC, T], bf16)
                xl_t = pA.tile([128, DC, T], bf16)
                for c in range(DC):
                    nc.sync.dma_start(out=xh_t[:, c, :], in_=xTh[c * 128:(c + 1) * 128, :])
                    nc.scalar.dma_start(out=xl_t[:, c, :], in_=xTl[c * 128:(c + 1) * 128, :])
                wgh_t = pA.tile([128, DC, 3], bf16)
                nc.sync.dma_start(out=wgh_t, in_=Wgh[:, :].rearrange("(c p) n -> p c n", p=128))
                wgl_t = pA.tile([128, DC, 3], bf16)
                nc.sync.dma_start(out=wgl_t, in_=Wgl[:, :].rearrange("(c p) n -> p c n", p=128))
                if with_bias:
                    bgh_t = pA.tile([1, 3], bf16)
                    nc.sync.dma_start(out=bgh_t, in_=bgh[:, :])
                    bgl_t = pA.tile([1, 3], bf16)
                    nc.sync.dma_start(out=bgl_t, in_=bgl[:, :])
                    xon = pA.tile([1, T], bf16)
                    nc.vector.memset(xon, 1.0)
                    bexph_t = pA.tile([3, D], bf16)
                    nc.sync.dma_start(out=bexph_t, in_=bexph[:, :])
                    bexpl_t = pA.tile([3, D], bf16)
                    nc.sync.dma_start(out=bexpl_t, in_=bexpl[:, :])

                # ---- gate logits, token layout [128 tok, 3] per tb ----
                glog = pA.tile([128, NTB, 3], f32)
                with tc.tile_pool(name="gps", bufs=4, space="PSUM") as gps:
                    for tb in range(NTB):
                        g_ps = gps.tile([128, 3], f32, tag="g")
                        first = True
                        for si, (xs, ws) in enumerate(((xh_t, wgh_t), (xh_t, wgl_t), (xl_t, wgh_t))):
                            for c in range(DC):
                                last = (not with_bias) and si == 2 and c == DC - 1
                                nc.tensor.matmul(
                                    out=g_ps[:, :],
                                    lhsT=xs[:, c, tb * 128:(tb + 1) * 128],
                                    rhs=ws[:, c, :],
                                    start=first, stop=last)
                                first = False
                        if with_bias:
                            nc.tensor.matmul(out=g_ps[:, :], lhsT=xon[0:1, tb * 128:(tb + 1) * 128],
                                             rhs=bgh_t[0:1, :], start=False, stop=False)
                            nc.tensor.matmul(out=g_ps[:, :], lhsT=xon[0:1, tb * 128:(tb + 1) * 128],
                                             rhs=bgl_t[0:1, :], start=False, stop=True)
                        nc.scalar.copy(out=glog[:, tb, :], in_=g_ps[:, :])

                # ---- gate softmax: shift by max, DVE polynomial exp ----
                cent = pA.tile([128, NTB, 3], f32)
                for tb in range(NTB):
                    nm = pA.tile([128, 1], f32, tag="gnm")
                    nc.vector.tensor_reduce(out=nm, in_=glog[:, tb, :], axis=AX,
                                            op=OP.max, negate=True)
                    nc.vector.tensor_scalar(out=cent[:, tb, :], in0=glog[:, tb, :],
                                            scalar1=nm, scalar2=None, op0=OP.add)
                flat = cent[:, :, :].rearrange("p a b -> p (a b)")
                u = pA.tile([128, NTB * 3], f32)
                # u = max(x, -16) * 0.25 + 2
                nc.vector.tensor_scalar(out=u, in0=flat, scalar1=-16.0, scalar2=0.25,
                                        op0=OP.max, op1=OP.mult)
                nc.vector.tensor_scalar(out=u, in0=u, scalar1=2.0, scalar2=None, op0=OP.add)
                pv = pA.tile([128, NTB * 3], f32)
                nc.vector.memset(pv, 0.0)
                for k in range(15, 0, -1):
                    nc.vector.scalar_tensor_tensor(out=pv, in0=pv, scalar=float(_EXP_C[k]),
                                                   in1=u, op0=OP.add, op1=OP.mult)
                nc.vector.tensor_scalar(out=pv, in0=pv, scalar1=float(_EXP_C[0]),
                                        scalar2=None, op0=OP.add)
                nc.vector.tensor_tensor(out=pv, in0=pv, in1=pv, op=OP.mult)
                nc.vector.tensor_tensor(out=pv, in0=pv, in1=pv, op=OP.mult)
                gexp = pv.rearrange("p (a b) -> p a b", b=3)

                # row sums + newton reciprocal -> rs [128, NTB]
                gs = pA.tile([128, NTB], f32)
                nc.vector.tensor_reduce(out=gs, in_=gexp, axis=AX, op=OP.add)
                rs0 = pA.tile([128, NTB], f32)
                nc.vector.reciprocal(out=rs0, in_=gs)
                tnw = pA.tile([128, NTB], f32)
                nc.vector.tensor_tensor(out=tnw, in0=gs, in1=rs0, op=OP.mult)
                nc.vector.tensor_scalar(out=tnw, in0=tnw, scalar1=2.0, scalar2=-1.0,
                                        op0=OP.subtract, op1=OP.mult)
                rs = pA.tile([128, NTB], f32)
                nc.vector.tensor_tensor(out=rs, in0=rs0, in1=tnw, op=OP.mult)

                # ---- broadcast g0/g1/g2 and 1/s across partitions ----
                gb_sb = [pA.tile([128, T], f32, tag=f"gb{e}", name=f"gb{e}") for e in range(3)]
                rb_sb = pA.tile([128, T], f32, tag="rb")
                with tc.tile_pool(name="bps", bufs=4, space="PSUM") as bps:
                    for tb in range(NTB):
                        for e in range(3):
                            t_ps = bps.tile([128, 128], f32, tag="bc")
                            nc.tensor.transpose(
                                out=t_ps[:, :],
                                in_=gexp[:, tb, e:e + 1].to_broadcast([128, 128]),
                                identity=ident[:, :])
                            nc.scalar.copy(out=gb_sb[e][:, tb * 128:(tb + 1) * 128], in_=t_ps)
                        t_ps = bps.tile([128, 128], f32, tag="bc")
                        nc.tensor.transpose(
                            out=t_ps[:, :],
                            in_=rs[:, tb:tb + 1].to_broadcast([128, 128]),
                            identity=ident[:, :])
                        nc.scalar.copy(out=rb_sb[:, tb * 128:(tb + 1) * 128], in_=t_ps)

                # ---- experts (transposed layout) + gated combine ----
                enhT = pA.tile([128, DC, T], f32)
                with tc.tile_pool(name="pw", bufs=2) as pw, \
                     tc.tile_pool(name="eps", bufs=2, space="PSUM") as eps:
                    for co in range(DC):
                        wt = {}
                        for e in range(3):
                            wt[(e, "h")] = pw.tile([128, DC, 128], bf16, tag=f"w{e}h", name=f"w{e}h")
                            nc.sync.dma_start(
                                out=wt[(e, "h")],
                                in_=Wh[e][:, co * 128:(co + 1) * 128].rearrange("(c p) n -> p c n", p=128))
                            wt[(e, "l")] = pw.tile([128, DC, 128], bf16, tag=f"w{e}l", name=f"w{e}l")
                            nc.scalar.dma_start(
                                out=wt[(e, "l")],
                                in_=Wl[e][:, co * 128:(co + 1) * 128].rearrange("(c p) n -> p c n", p=128))
                        for half in range(2):
                            hs = slice(half * 512, (half + 1) * 512)
                            e_ps = []
                            for e in range(3):
                                ps_ = eps.tile([128, 512], f32, tag=f"e{e}")
                                first = True
                                for (xs, wk) in ((xh_t, "h"), (xh_t, "l"), (xl_t, "h")):
                                    for c in range(DC):
                                        last = (not with_bias) and xs is xl_t and c == DC - 1
                                        nc.tensor.matmul(
                                            out=ps_[:, :], lhsT=wt[(e, wk)][:, c, :],
                                            rhs=xs[:, c, hs], start=first, stop=last)
                                        first = False
                                if with_bias:
                                    nc.tensor.matmul(
                                        out=ps_[:, :], lhsT=bexph_t[e:e + 1, co * 128:(co + 1) * 128],
                                        rhs=xon[0:1, hs], start=False, stop=False)
                                    nc.tensor.matmul(
                                        out=ps_[:, :], lhsT=bexpl_t[e:e + 1, co * 128:(co + 1) * 128],
                                        rhs=xon[0:1, hs], start=False, stop=True)
                                e_ps.append(ps_)
                            uacc = pA.tile([128, 512], f32, tag="uacc")
                            vtmp = pA.tile([128, 512], f32, tag="vtmp")
                            nc.vector.tensor_tensor(out=uacc, in0=e_ps[0], in1=gb_sb[0][:, hs], op=OP.mult)
                            nc.vector.tensor_tensor(out=vtmp, in0=e_ps[1], in1=gb_sb[1][:, hs], op=OP.mult)
                            nc.vector.tensor_tensor(out=uacc, in0=uacc, in1=vtmp, op=OP.add)
                            nc.vector.tensor_tensor(out=vtmp, in0=e_ps[2], in1=gb_sb[2][:, hs], op=OP.mult)
                            nc.vector.tensor_tensor(out=uacc, in0=uacc, in1=vtmp, op=OP.add)
                            nc.vector.tensor_tensor(out=uacc, in0=uacc, in1=rb_sb[:, hs], op=OP.mult)
                            nc.vector.tensor_scalar(out=enhT[:, co, hs], in0=uacc,
                                                    scalar1=vbar_t[:, co:co + 1], scalar2=None,
                                                    op0=OP.add)
                        # split this chunk of enhancedT into bf16 hi/lo right
                        # away so phase B can start as soon as all chunks land
                        nc.vector.tensor_copy(out=EhT[:, co, :], in_=enhT[:, co, :])
                        nc.vector.tensor_tensor(out=ElT[:, co, :], in0=enhT[:, co, :],
                                                in1=EhT[:, co, :], op=OP.subtract)

            # =========================== phase B ===========================
            with tc.tile_pool(name="pB", bufs=1) as pB, \
                 tc.tile_pool(name="pBs", bufs=3) as pBs, \
                 tc.tile_pool(name="lps", bufs=6, space="PSUM") as lps, \
                 tc.tile_pool(name="qps", bufs=2, space="PSUM") as qps:
                NSLOT = 4  # ring of exp slots (tb % 4)
                expbuf = pB.tile([128, NSLOT, KCB], f16)
                logit_dmas = {tb: [] for tb in range(NTB)}

                def sweep(sw, fc_sched=None):
                    tbs = SWEEP_TBS[sw]
                    for nb in range(NNB):
                        if fc_sched and nb in fc_sched:
                            for _tb in fc_sched[nb]:
                                finalize(_tb)
                        ns = slice(nb * 512, (nb + 1) * 512)
                        wlh_t = pWL.tile([128, DC, 512], bf16, tag="wlh")
                        nc.sync.dma_start(out=wlh_t, in_=WLh[:, ns].rearrange("(c p) n -> p c n", p=128))
                        wll_t = pWL.tile([128, DC, 512], bf16, tag="wll")
                        nc.scalar.dma_start(out=wll_t, in_=WLl[:, ns].rearrange("(c p) n -> p c n", p=128))
                        if with_bias:
                            blh_t = pBs.tile([1, 512], bf16, tag="blh")
                            nc.sync.dma_start(out=blh_t, in_=blh[:, ns])
                            bll_t = pBs.tile([1, 512], bf16, tag="bll")
                            nc.sync.dma_start(out=bll_t, in_=bll[:, ns])
                        for tb in tbs:
                            ts_ = slice(tb * 128, (tb + 1) * 128)
                            lp = lps.tile([128, 512], f32, tag="lp")
                            first = True
                            for (es, ws) in ((EhT, wlh_t), (EhT, wll_t), (ElT, wlh_t)):
                                for c in range(DC):
                                    last = (not with_bias) and es is ElT and c == DC - 1
                                    nc.tensor.matmul(out=lp[:, :], lhsT=es[:, c, ts_],
                                                     rhs=ws[:, c, :], start=first, stop=last)
                                    first = False
                            if with_bias:
                                nc.tensor.matmul(out=lp[:, :], lhsT=ones_h[0:1, ts_],
                                                 rhs=blh_t[0:1, :], start=False, stop=False)
                                nc.tensor.matmul(out=lp[:, :], lhsT=ones_h[0:1, ts_],
                                                 rhs=bll_t[0:1, :], start=False, stop=True)
                            stg = pBs.tile([128, 512], f32, tag="lstg")
                            nc.scalar.copy(out=stg, in_=lp)
                            dmi = nc.scalar.dma_start(out=logits_o[ts_, ns], in_=stg)
                            logit_dmas[tb].append(dmi)
                            nc.scalar.activation(
                                out=expbuf[:, tb % NSLOT, ns], in_=lp, func=AF.Exp,
                                bias=0.0, scale=1.0,
                                accum_out=spart[:, tb, nb:nb + 1])
                            nc.vector.tensor_reduce(out=mpart[:, tb, nb:nb + 1], in_=lp,
                                                    axis=AX, op=OP.max)

                def finalize(tb):
                    ts_ = slice(tb * 128, (tb + 1) * 128)
                    lt = tb % NSLOT
                    # softmax scale r = 1/S (newton)
                    s1 = pBs.tile([128, 1], f32, tag="s1")
                    nc.vector.tensor_reduce(out=s1, in_=spart[:, tb, :], axis=AX, op=OP.add)
                    r0 = pBs.tile([128, 1], f32, tag="r0")
                    nc.vector.reciprocal(out=r0, in_=s1)
                    tn = pBs.tile([128, 1], f32, tag="tn")
                    nc.vector.tensor_tensor(out=tn, in0=s1, in1=r0, op=OP.mult)
                    nc.vector.tensor_scalar(out=tn, in0=tn, scalar1=2.0, scalar2=-1.0,
                                            op0=OP.subtract, op1=OP.mult)
                    rr = pBs.tile([128, 1], f32, tag="rr")
                    nc.vector.tensor_tensor(out=rr, in0=r0, in1=tn, op=OP.mult)
                    # soft = expbuf * r first: frees this tb's exp slot ASAP
                    for ch in range(4):
                        cs = slice(ch * 2048, (ch + 1) * 2048)
                        sst = pBs.tile([128, 2048], f32, tag="sst", name="sst", bufs=2)
                        nc.vector.tensor_scalar(out=sst, in0=expbuf[:, lt, cs],
                                                scalar1=rr, scalar2=None, op0=OP.mult)
                        nc.sync.dma_start(out=soft_o[ts_, cs], in_=sst)
                    # argmax: row max + winning block
                    mx = pBs.tile([128, 1], f32, tag="mx")
                    nc.vector.tensor_reduce(out=mx, in_=mpart[:, tb, :], axis=AX, op=OP.max)
                    bsl = pBs.tile([128, NNB], f32, tag="bsl")
                    nc.vector.scalar_tensor_tensor(out=bsl, in0=mpart[:, tb, :], scalar=mx,
                                                   in1=iota16, op0=OP.is_equal, op1=OP.mult)
                    bv = pBs.tile([128, 1], f32, tag="bv")
                    nc.vector.tensor_reduce(out=bv, in_=bsl, axis=AX, op=OP.max)
                    bstar = pBs.tile([128, 1], f32, tag="bstar")
                    nc.vector.tensor_scalar(out=bstar, in0=bv, scalar1=float(NNB - 1),
                                            scalar2=-1.0, op0=OP.subtract, op1=OP.mult)
                    # gather the winning 512-block of this tb's logits rows
                    rrow = pBs.tile([128, 1], f32, tag="rrow")
                    nc.vector.scalar_tensor_tensor(out=rrow, in0=tokio[:, tb:tb + 1],
                                                   scalar=float(NNB), in1=bstar,
                                                   op0=OP.mult, op1=OP.add)
                    ri = pBs.tile([128, 1], i32, tag="ri")
                    nc.vector.tensor_copy(out=ri, in_=rrow)
                    gl = pBs.tile([128, 512], f32, tag="gl")
                    gth = nc.gpsimd.indirect_dma_start(
                        out=gl[:, :], out_offset=None, in_=logits_rows,
                        in_offset=bass.IndirectOffsetOnAxis(ap=ri[:, :1], axis=0))
                    for dmi in logit_dmas[tb]:
                        add_dep_helper(gth.ins, dmi.ins, reason="gather logits after writeback")
                    psl = pBs.tile([128, 512], f32, tag="psl")
                    nc.vector.scalar_tensor_tensor(out=psl, in0=gl, scalar=mx, in1=iota512,
                                                   op0=OP.is_equal, op1=OP.mult)
                    pv_ = pBs.tile([128, 1], f32, tag="pv_")
                    nc.vector.tensor_reduce(out=pv_, in_=psl, axis=AX, op=OP.max)
                    pos = pBs.tile([128, 1], f32, tag="pos")
                    nc.vector.tensor_scalar(out=pos, in0=pv_, scalar1=511.0, scalar2=-1.0,
                                            op0=OP.subtract, op1=OP.mult)
                    idxf = pBs.tile([128, 1], f32, tag="idxf")
                    nc.vector.scalar_tensor_tensor(out=idxf, in0=bstar, scalar=512.0,
                                                   in1=pos, op0=OP.mult, op1=OP.add)
                    nc.sync.dma_start(out=idx_o[tb:tb + 1, :], in_=idxf[:, 0:1])
                    idxi = pBs.tile([128, 1], i32, tag="idxi")
                    nc.vector.tensor_copy(out=idxi, in_=idxf)
                    # quantized = codebook[idx]
                    qg = pBs.tile([128, D], f32, tag="qg")
                    nc.gpsimd.indirect_dma_start(
                        out=qg[:, :], out_offset=None, in_=cb[:, :],
                        in_offset=bass.IndirectOffsetOnAxis(ap=idxi[:, :1], axis=0))
                    nc.gpsimd.dma_start(out=quant_o[ts_, :], in_=qg)
                    # vq loss partials: sum_d (q - enh)^2 in transposed layout
                    for c in range(DC):
                        qt_ps = qps.tile([128, 128], f32, tag="qt")
                        nc.tensor.matmul(out=qt_ps[:, :], lhsT=qg[:, c * 128:(c + 1) * 128],
                                         rhs=ident[:, :], start=True, stop=True)
                        df = pBs.tile([128, 128], f32, tag="df")
                        nc.vector.tensor_tensor(out=df, in0=qt_ps, in1=EhT[:, c, ts_], op=OP.subtract)
                        nc.vector.tensor_tensor(out=df, in0=df, in1=ElT[:, c, ts_], op=OP.subtract)
                        sqs = pBs.tile([128, 128], f32, tag="sqs")
                        nc.scalar.activation(out=sqs, in_=df, func=AF.Square,
                                             bias=0.0, scale=1.0,
                                             accum_out=lossp[:, tb * DC + c:tb * DC + c + 1])

                sweep(0)
                # interleave earlier sweeps' finalizes into later sweeps.
                # a finalize must be emitted before the first tile write of
                # any sweep that reuses its exp ring slot (slot = tb % 4):
                # sweep1 (tb 3,4,5 -> slots 3,0,1) reuses tb0/tb1's slots;
                # sweep2 (tb 6,7 -> slots 2,3) reuses tb2/tb3's slots.
                sweep(1, {0: [0, 1], 8: [2]})
                sweep(2, {0: [3], 5: [4], 10: [5]})
                for tb in SWEEP_TBS[2]:
                    finalize(tb)

                lsum = pBs.tile([128, 1], f32, tag="lsum")
                nc.vector.tensor_reduce(out=lsum, in_=lossp, axis=AX, op=OP.add)
                nc.sync.dma_start(out=loss_o[:, :], in_=lsum)
            pWL_cm.__exit__(None, None, None)

    nc.finalize()
    return nc


# ----------------------------- host side -----------------------------

_NC_CACHE = {}


def _get_nc(with_bias: bool):
    if with_bias not in _NC_CACHE:
        _NC_CACHE[with_bias] = build_nc(with_bias)
    return _NC_CACHE[with_bias]


def _split(a):
    a = np.ascontiguousarray(a, dtype=np.float32)
    h = a.astype(_BF)
    l = (a - h.astype(np.float32)).astype(_BF)
    return h, l


def _prep(inputs):
    x = np.asarray(inputs["x"], np.float32)
    with_bias = any(
        np.any(np.asarray(inputs[k])) for k in ("b_low", "b_mid", "b_high", "b_gate", "b_logits"))

    W = {}
    for e, nm in enumerate(("W_low", "W_mid", "W_high")):
        h, l = _split(np.asarray(inputs[nm]))
        W[f"W{e}h"] = h
        W[f"W{e}l"] = l
    W["Wgh"], W["Wgl"] = _split(np.asarray(inputs["W_gate"]))
    W["WLh"], W["WLl"] = _split(np.asarray(inputs["W_logits"]))
    vb = np.asarray(inputs["vehicle_emb"], np.float32).mean(axis=0)
    W["vbar"] = np.ascontiguousarray(vb.reshape(DC, 128).T, np.float32)
    W["cb"] = np.ascontiguousarray(np.asarray(inputs["codebook"], np.float32))
    if with_bias:
        bh, bl = _split(np.stack([np.asarray(inputs[k], np.float32)
                                  for k in ("b_low", "b_mid", "b_high")]))
        W["bexph"], W["bexpl"] = bh, bl
        gh, gl = _split(np.asarray(inputs["b_gate"], np.float32)[None, :])
        W["bgh"], W["bgl"] = gh, gl
        lh, ll = _split(np.asarray(inputs["b_logits"], np.float32)[None, :])
        W["blh"], W["bll"] = lh, ll

    in_maps = []
    for c in range(B):
        xT = np.ascontiguousarray(x[c].T)
        xh, xl = _split(xT)
        m = {"xTh": xh, "xTl": xl}
        m.update(W)
        in_maps.append(m)
    return in_maps, with_bias


def _run(inputs, trace=False):
    from concourse.bass_utils import run_bass_kernel_spmd
    in_maps, with_bias = _prep(inputs)
    nc = _get_nc(with_bias)
    res = run_bass_kernel_spmd(nc, in_maps, list(range(B)), trace=trace)
    return res


def _assemble(res):
    quant = np.stack([res.results[c]["quant_o"] for c in range(B)])
    logits = np.stack([res.results[c]["logits_o"] for c in range(B)])
    soft = np.stack([res.results[c]["soft_o"] for c in range(B)])
    idx = np.stack([np.rint(res.results[c]["idx_o"].reshape(T)).astype(np.int32)
                    for c in range(B)])
    tot = sum(float(res.results[c]["loss_o"].sum(dtype=np.float64)) for c in range(B))
    vq_loss = np.float32(tot / (B * T * D))
    return quant, idx, vq_loss, soft, logits


def kernel(**inputs):
    res = _run(inputs, trace=False)
    return _assemble(res)


def model_time_ns(with_bias=False):
    """Cost-model execution time (no NTFF profiling available under this
    axon container, so CoreSim's instruction cost model is the ns source)."""
    import concourse.bass_interp as bass_interp
    nc = _get_nc(with_bias)
    sim = bass_interp.CoreSim(nc, no_exec=True, publish_trace=False)
    sim.simulate()
    return int(sim.time)


def kernel_profiled(**inputs):
    res = _run(inputs, trace=False)
    out = _assemble(res)
    return out, model_time_ns()
